# revision 1
# baseline (speedup 1.0000x reference)
"""Trainium2 Bass kernel for nn_Connection_75411035783724 (Mamba2 block + MLP head).

Sharding: tensor-parallel over the 32 Mamba2 heads across 8 cores (4 heads each).
Each core computes the in_proj column-slice it needs (its x-channels + B + dt),
the depthwise causal conv (as 4 accumulating diagonal matmuls on the PE),
and a chunked-SSD scan (chunk 256): per-chunk states via PE matmuls over
transposed activations, inter-chunk recurrence via a single tensor_tensor_scan.
Only the 32 frame-start tokens are ever projected to the output (the reference
discards all other rows), so the gated RMSNorm + out_proj + MLP run on 32 rows
only (launch 2, column-sharded MLP2).

Key numeric choices: bf16 for matmul operands and big intermediates, fp32 PSUM
accumulation and fp32 for the dt/decay pipeline.
"""
import os
import sys
import numpy as np
import ml_dtypes

sys.path.insert(0, "/opt/trn_rl_repo")

import concourse.bass as bass
import concourse.tile as tile
import concourse.mybir as mybir
from concourse import bacc
from concourse import bass_utils

F32 = mybir.dt.float32
BF16 = mybir.dt.bfloat16
AF = mybir.ActivationFunctionType
OP = mybir.AluOpType
BF = ml_dtypes.bfloat16

# Model dims
D_MODEL = 1024
HIDDEN = 4096
D_STATE = 128       # n
D_CONV = 4
D_INNER = 2048
HEADDIM = 64        # p
NHEADS = 32
CONV_DIM = D_INNER + 2 * D_STATE            # 2304
D_IN_PROJ = 2 * D_INNER + 2 * D_STATE + NHEADS  # 4384
L = 8192            # tokens
NPOS = 32           # output positions (first token of each frame)
POS_STRIDE = 256
NCORES = 8
HPC = 4             # heads per core
Q = 256             # chunk length
NCHUNK = L // Q     # 32
KT = D_MODEL // 128  # 8 K-tiles
NG = 16             # token groups of 512
GSZ = 512
# in_proj col slice per core: [x 256 | B 128]; dt handled via its own tensor
NCOL = 256 + 128  # 384
MT_SPEC = [(0, 128), (128, 128), (256, 128)]  # (col0, width)


def _bf(x):
    return np.ascontiguousarray(np.asarray(x, dtype=np.float32)).astype(BF)


def _f32(x):
    return np.ascontiguousarray(np.asarray(x, dtype=np.float32))


# ----------------------------------------------------------------------------
# Launch 1 program: in_proj + conv + scan -> gated y at the 32 positions
# ----------------------------------------------------------------------------
_L1 = None


def build_l1():
    global _L1
    if _L1 is not None:
        return _L1
    nc = bacc.Bacc("TRN2", target_bir_lowering=False, debug=False,
                   num_devices=NCORES)

    def din(name, shape, dt):
        return nc.dram_tensor(name, shape, dt, kind="ExternalInput").ap()

    xT = din("xT", (D_MODEL, L), BF16)
    xTpos = din("xTpos", (D_MODEL, NPOS), BF16)
    xTwin = din("xTwin", (D_MODEL, NPOS * D_CONV), BF16)
    w_in = din("w_in", (KT, 128, NCOL), BF16)
    w_dt = din("w_dt", (KT, 128, HPC), BF16)
    w_c = din("w_c", (KT, 128, 128), BF16)
    w_z = din("w_z", (KT, 128, 256), BF16)
    diag_w = din("diag_w", (3, D_CONV, 128, 128), BF16)
    cw_c = din("cw_c", (128, D_CONV), F32)
    conv_b = din("conv_b", (128, 3), F32)
    conv_b_c = din("conv_b_c", (128, 1), F32)
    dtb4 = din("dtb4", (HPC, 1), F32)
    A4 = din("A4", (HPC, 1), F32)
    D4 = din("D4", (HPC, 1), F32)
    y32g_out = nc.dram_tensor("y32g", (128, 2, NPOS), F32,
                              kind="ExternalOutput").ap()

    BSZ = 2 * GSZ          # 1024-token batches for the decay pipe
    NB = L // BSZ          # 8
    CPB = BSZ // Q         # 4 chunks per batch

    with tile.TileContext(nc) as tc:
        import contextlib
        with contextlib.ExitStack() as ctx:
            sb = ctx.enter_context(tc.tile_pool(name="sb", bufs=1))
            ring = ctx.enter_context(tc.tile_pool(name="ring", bufs=1))
            dsc = ctx.enter_context(tc.tile_pool(name="dsc", bufs=1, space="DRAM"))
            psA = ctx.enter_context(tc.tile_pool(name="psA", bufs=1, space="PSUM"))

            # ---- resident weights/constants
            w_in_sb = sb.tile([128, KT, NCOL], BF16)
            nc.sync.dma_start(out=w_in_sb, in_=w_in.rearrange("k p c -> p k c"))
            w_dt_sb = sb.tile([128, KT, HPC], BF16)
            nc.sync.dma_start(out=w_dt_sb, in_=w_dt.rearrange("k p c -> p k c"))
            w_c_sb = sb.tile([128, KT, 128], BF16)
            nc.sync.dma_start(out=w_c_sb, in_=w_c.rearrange("k p c -> p k c"))
            w_z_sb = sb.tile([128, KT, 256], BF16)
            nc.sync.dma_start(out=w_z_sb, in_=w_z.rearrange("k p c -> p k c"))
            diag_sb = sb.tile([128, 3, D_CONV, 128], BF16)
            nc.sync.dma_start(out=diag_sb, in_=diag_w.rearrange("c j a b -> a c j b"))
            cw_sb = sb.tile([128, D_CONV], F32)
            nc.sync.dma_start(out=cw_sb, in_=cw_c)
            cb_sb = sb.tile([128, 3], F32)
            nc.sync.dma_start(out=cb_sb, in_=conv_b)
            cbc_sb = sb.tile([128, 1], F32)
            nc.sync.dma_start(out=cbc_sb, in_=conv_b_c)
            dtb_sb = sb.tile([HPC, 1], F32)
            nc.sync.dma_start(out=dtb_sb, in_=dtb4)
            A_sb = sb.tile([HPC, 1], F32)
            nc.sync.dma_start(out=A_sb, in_=A4)
            D_sb = sb.tile([HPC, 1], F32)
            nc.sync.dma_start(out=D_sb, in_=D4)
            xtp_sb = sb.tile([128, KT, NPOS], BF16)
            nc.sync.dma_start(out=xtp_sb, in_=xTpos.rearrange("(k p) t -> p k t", p=128))
            xtw_sb = sb.tile([128, KT, NPOS * D_CONV], BF16)
            nc.sync.dma_start(out=xtw_sb, in_=xTwin.rearrange("(k p) t -> p k t", p=128))

            # ---- persistent big buffers
            xbcc = sb.tile([128, 3, L], BF16, tag="big1")  # conv+silu [x0|x1|B]
            XT = sb.tile([128, L // 128, 256], BF16, tag="big2")  # transposed x*w
            BT = sb.tile([128, L // 128, 128], BF16, tag="bt")    # transposed B
            S_all = sb.tile([128, HPC * HEADDIM * NCHUNK], BF16)  # (h,p,c)
            lamA = sb.tile([HPC, NCHUNK], F32)       # per-chunk decay
            dAA = sb.tile([HPC, NCHUNK], F32)        # exp(a) at chunk starts
            dt_pos = sb.tile([HPC, NCHUNK], F32)     # dt at chunk starts
            x32 = sb.tile([128, 2, NPOS], F32)
            B32 = sb.tile([128, NPOS], F32)
            # narrow work tiles (all base partition 0 -- engine ops require
            # both operands on the same partition set)
            ones4 = sb.tile([HPC, BSZ], F32)
            dt2_slots = [sb.tile([HPC, BSZ], F32, tag="dt2a", name="dt2a"),
                         sb.tile([HPC, BSZ], F32, tag="dt2b", name="dt2b")]
            tmp_s = sb.tile([HPC, BSZ], F32)
            s2_s = sb.tile([HPC, BSZ], F32)
            w2b_pack = sb.tile([HPC, BSZ], BF16)
            nc.vector.memset(ones4, 1.0)
            # zero at chunk starts -> cumsum resets there
            z_ap = bass.AP(tensor=ones4.tensor, offset=ones4.offset,
                           ap=[list(ones4.ap[0]), [Q, CPB]])
            nc.vector.memset(z_ap, 0.0)

            # ================= main fused loop =================
            w_dd = dsc.tile([HPC, L], BF16)
            prev_xbc = None
            xt_h = None
            dt_ps = []
            for g in range(NG):
                sl = slice(g * GSZ, (g + 1) * GSZ)
                if g % 2 == 0:
                    xt_h = ring.tile([128, KT, BSZ], BF16, tag="xt", bufs=1)
                    hsl = slice(g * GSZ, (g + 2) * GSZ)
                    nc.sync.dma_start(
                        out=xt_h,
                        in_=xT.rearrange("(k p) t -> p k t", p=128)[:, :, hsl])
                xt_g = xt_h[:, :, (g % 2) * GSZ:(g % 2 + 1) * GSZ]
                # in_proj matmuls: x0, x1, B cols + dt
                ps = []
                for mt, (c0, cw) in enumerate(MT_SPEC):
                    p = psA.tile([cw, GSZ], F32, tag=f"pin{mt}")
                    for k in range(KT):
                        nc.tensor.matmul(p, w_in_sb[:, k, c0:c0 + cw],
                                         xt_g[:, k, :],
                                         start=(k == 0), stop=(k == KT - 1))
                    ps.append(p)
                pdt = psA.tile([HPC, GSZ], F32, tag="pdt", bufs=2)
                for k in range(KT):
                    nc.tensor.matmul(pdt, w_dt_sb[:, k, :], xt_g[:, k, :],
                                     start=(k == 0), stop=(k == KT - 1))
                dt_ps.append(pdt)
                # evict to xbc ring (3 halo cols at the front)
                xbc_g = ring.tile([128, 3, GSZ + 3], BF16, tag="xbc", bufs=2)
                if prev_xbc is None:
                    nc.vector.memset(xbc_g[:, :, 0:3], 0.0)
                else:
                    nc.vector.tensor_copy(out=xbc_g[:, :, 0:3],
                                          in_=prev_xbc[:, :, GSZ:GSZ + 3])
                for cht in range(3):
                    if cht != 2:
                        nc.vector.tensor_copy(out=xbc_g[:, cht, 3:], in_=ps[cht])
                    else:
                        nc.scalar.copy(out=xbc_g[:, cht, 3:], in_=ps[cht])
                prev_xbc = xbc_g
                # conv: 4 accumulating diag matmuls per channel tile + silu evict
                for cht in range(3):
                    pc = psA.tile([128, GSZ], F32, tag="pcv", bufs=2)
                    for j in range(D_CONV):
                        nc.tensor.matmul(pc, diag_sb[:, cht, j, :],
                                         xbc_g[:, cht, j:j + GSZ],
                                         start=(j == 0), stop=(j == D_CONV - 1))
                    nc.scalar.activation(out=xbcc[:, cht, sl], in_=pc,
                                         func=AF.Silu,
                                         bias=cb_sb[:, cht:cht + 1], scale=1.0)
                if g % 4 == 3:
                    qsl = slice((g - 3) * GSZ, (g + 1) * GSZ)
                    nc.scalar.dma_start_transpose(
                        out=BT[:, 4 * (g - 3):4 * (g + 1), :],
                        in_=xbcc[:, 2, qsl])
                if g % 2 == 0:
                    continue
                # ======= per-2-group decay pipe + scale + transpose + scan ====
                b = g // 2
                bsl = slice(b * BSZ, (b + 1) * BSZ)
                dt2 = dt2_slots[b % 2]
                nc.vector.tensor_copy(out=dt2[:, 0:GSZ], in_=dt_ps[-2])
                nc.vector.tensor_copy(out=dt2[:, GSZ:], in_=dt_ps[-1])
                dt_ps.clear()
                # softplus(v) = v - ln(sigmoid(v))
                nc.vector.tensor_scalar(dt2, dt2, dtb_sb[:, 0:1], None, OP.add)
                tmp = tmp_s
                nc.scalar.activation(out=tmp, in_=dt2, func=AF.Sigmoid)
                nc.scalar.activation(out=tmp, in_=tmp, func=AF.Ln)
                nc.vector.tensor_sub(dt2, dt2, tmp)
                # a = dt*A ; per-chunk inclusive cumsum (reset via zeroed mask)
                a2 = tmp_s
                nc.vector.tensor_scalar_mul(a2, dt2, A_sb[:, 0:1])
                s2 = s2_s
                nc.vector.tensor_tensor_scan(out=s2, data0=ones4, data1=a2,
                                             initial=0.0, op0=OP.mult, op1=OP.add)
                # extracts at chunk starts / ends
                cpos = b * CPB
                src = bass.AP(tensor=a2.tensor, offset=a2.offset,
                              ap=[list(a2.ap[0]), [Q, CPB]])
                nc.scalar.activation(out=dAA[:, cpos:cpos + CPB], in_=src, func=AF.Exp)
                src = bass.AP(tensor=dt2.tensor, offset=dt2.offset,
                              ap=[list(dt2.ap[0]), [Q, CPB]])
                nc.vector.tensor_copy(out=dt_pos[:, cpos:cpos + CPB], in_=src)
                src = bass.AP(tensor=s2.tensor, offset=s2.offset + Q - 1,
                              ap=[list(s2.ap[0]), [Q, CPB]])
                nc.scalar.activation(out=lamA[:, cpos:cpos + CPB], in_=src, func=AF.Exp)
                # w = exp(stot - s) * dt
                for cc in range(CPB):
                    stot = bass.AP(tensor=s2.tensor,
                                   offset=s2.offset + cc * Q + Q - 1,
                                   ap=[list(s2.ap[0]), [1, 1]])
                    nc.vector.tensor_scalar(s2[:, cc * Q:(cc + 1) * Q],
                                            s2[:, cc * Q:(cc + 1) * Q],
                                            stot, None, OP.subtract)
                w2b = w2b_pack
                nc.scalar.activation(out=s2, in_=s2, func=AF.Exp, scale=-1.0)
                nc.vector.tensor_mul(s2, s2, dt2)
                nc.vector.tensor_copy(out=w2b, in_=s2)
                # DRAM roundtrip broadcast: (h, tok-slice) -> [(hh,p), t, tok]
                nc.gpsimd.dma_start(out=w_dd[:, bsl], in_=w2b)
                wbc2 = ring.tile([128, 2, BSZ], BF16, tag="wbc", bufs=1)
                for t in range(2):
                    src = bass.AP(tensor=w_dd.tensor,
                                  offset=w_dd.offset + (2 * t) * L + b * BSZ,
                                  ap=[[L, 2], [0, 64], [1, BSZ]])
                    nc.gpsimd.dma_start(out=wbc2[:, t, :], in_=src)
                xs2 = ring.tile([128, 2, BSZ], BF16, tag="xs", bufs=1)
                for t in range(2):
                    nc.vector.tensor_mul(xs2[:, t, :], xbcc[:, t, bsl],
                                         wbc2[:, t, :])
                    eng = nc.sync if t == 0 else nc.scalar
                    eng.dma_start_transpose(
                        out=XT[:, 8 * b:8 * (b + 1), t * 128:(t + 1) * 128],
                        in_=xs2[:, t, :])
                # scan matmuls: emitted at odd batches, after the BT transpose
                # covering these tokens (deps follow emission order)
                if b % 2 == 0:
                    continue
                for cc in range(2 * CPB):
                    c = (b - 1) * CPB + cc
                    pc2 = psA.tile([128, HPC * HEADDIM], F32, tag="psc", bufs=1)
                    for k2 in range(2):
                        T = 2 * c + k2
                        nc.tensor.matmul(pc2, BT[:, T, :], XT[:, T, :],
                                         start=(k2 == 0), stop=(k2 == 1))
                    dst = bass.AP(tensor=S_all.tensor, offset=S_all.offset + c,
                                  ap=[list(S_all.ap[0]),
                                      [HEADDIM * NCHUNK, HPC], [NCHUNK, HEADDIM]])
                    if c % 2 == 0:
                        nc.vector.tensor_copy(out=dst, in_=pc2.rearrange(
                            "n (h p) -> n h p", h=HPC))
                    else:
                        nc.scalar.copy(out=dst, in_=pc2.rearrange(
                            "n (h p) -> n h p", h=HPC))

            # ---- batched position extracts (pre-scale values live in xbcc? no:
            # x columns were scaled in place? -> no, scaling wrote xs2; xbcc intact)
            for cht in range(2):
                s_ap = bass.AP(tensor=xbcc.tensor, offset=xbcc.offset + cht * L,
                               ap=[list(xbcc.ap[0]), [POS_STRIDE, NPOS]])
                nc.vector.tensor_copy(out=x32[:, cht, :], in_=s_ap)
            s_ap = bass.AP(tensor=xbcc.tensor, offset=xbcc.offset + 2 * L,
                           ap=[list(xbcc.ap[0]), [POS_STRIDE, NPOS]])
            nc.vector.tensor_copy(out=B32, in_=s_ap)

            # ================= tail =================
            # C at positions (conv window matmul + 4-tap conv + silu)
            C32 = sb.tile([128, NPOS], F32)
            pcw = psA.tile([128, NPOS * D_CONV], F32, tag="psc")
            for k in range(KT):
                nc.tensor.matmul(pcw, w_c_sb[:, k, :], xtw_sb[:, k, :],
                                 start=(k == 0), stop=(k == KT - 1))
            tmpc = sb.tile([128, NPOS], F32)
            for j in range(D_CONV):
                src = bass.AP(tensor=pcw.tensor, offset=pcw.offset + j,
                              ap=[list(pcw.ap[0]), [D_CONV, NPOS]])
                if j == 0:
                    nc.vector.tensor_scalar_mul(tmpc, src, cw_sb[:, 0:1])
                else:
                    nc.vector.scalar_tensor_tensor(
                        out=tmpc, in0=src, scalar=cw_sb[:, j:j + 1], in1=tmpc,
                        op0=OP.mult, op1=OP.add)
            nc.scalar.activation(out=C32, in_=tmpc, func=AF.Silu,
                                 bias=cbc_sb[:, 0:1], scale=1.0)
            C32b = sb.tile([128, NPOS], BF16)
            nc.vector.tensor_copy(out=C32b, in_=C32)

            # z gate values at positions
            pz = psA.tile([128, 2, NPOS], F32, tag="psc")
            for t in range(2):
                for k in range(KT):
                    nc.tensor.matmul(pz[:, t, :],
                                     w_z_sb[:, k, t * 128:(t + 1) * 128],
                                     xtp_sb[:, k, :],
                                     start=(k == 0), stop=(k == KT - 1))
            zs = sb.tile([128, 2, NPOS], F32)
            nc.scalar.activation(out=zs, in_=pz, func=AF.Silu)

            # BC row = sum_n B32*C32 -> [1, NPOS]
            bc_t = sb.tile([128, NPOS], F32)
            nc.vector.tensor_mul(bc_t, B32, C32)
            ones1 = sb.tile([128, 1], F32)
            nc.vector.memset(ones1, 1.0)
            pbc = psA.tile([1, NPOS], F32, tag="psc")
            nc.tensor.matmul(pbc, ones1, bc_t, start=True, stop=True)
            bc_row = sb.tile([1, NPOS], F32)
            nc.vector.tensor_copy(out=bc_row, in_=pbc)
            # BC -> [HPC, NPOS] via DRAM roundtrip; f = dt*.BC + D
            bc_d = dsc.tile([1, NPOS], F32)
            nc.gpsimd.dma_start(out=bc_d, in_=bc_row)
            bc4 = sb.tile([HPC, NPOS], F32)
            nc.gpsimd.dma_start(
                out=bc4, in_=bass.AP(tensor=bc_d.tensor, offset=bc_d.offset,
                                     ap=[[0, HPC], [1, NPOS]]))
            f4 = sb.tile([HPC, NPOS], F32)
            nc.vector.tensor_mul(f4, dt_pos, bc4)
            nc.vector.tensor_scalar(f4, f4, D_sb[:, 0:1], None, OP.add)

            # broadcasts of dAA/f4 -> [128, 2, NPOS] via DRAM (tiny)
            small_d = dsc.tile([2, HPC, NCHUNK], F32)
            nc.gpsimd.dma_start(out=small_d[0], in_=dAA)
            nc.gpsimd.dma_start(out=small_d[1], in_=f4)

            def bcast_hp(col):
                t_sb = sb.tile([128, 2, NPOS], F32, tag=f"bch{col}")
                for t in range(2):
                    for hh in range(2):
                        src = bass.AP(
                            tensor=small_d.tensor,
                            offset=small_d.offset + col * HPC * NCHUNK
                            + (2 * t + hh) * NCHUNK,
                            ap=[[0, 64], [1, NPOS]])
                        nc.gpsimd.dma_start(
                            out=t_sb[64 * hh:64 * (hh + 1), t, :], in_=src)
                return t_sb

            dA_bc = bcast_hp(0)
            f_bc = bcast_hp(1)

            # lambda -> lam_flat [128, (h,p,c)] (DRAM expand + on-chip bcast)
            nc.vector.memset(lamA[:, 0:1], 0.0)
            lam16 = sb.tile([HPC, NCHUNK], BF16)
            nc.vector.tensor_copy(out=lam16, in_=lamA)
            lam_d = dsc.tile([HPC, NCHUNK], BF16)
            nc.gpsimd.dma_start(out=lam_d, in_=lam16)
            lam_row = sb.tile([1, HPC * HEADDIM * NCHUNK], BF16)
            lam_src = bass.AP(tensor=lam_d.tensor, offset=lam_d.offset,
                              ap=[[0, 1], [NCHUNK, HPC], [0, HEADDIM], [1, NCHUNK]])
            nc.gpsimd.dma_start(
                out=lam_row.rearrange("o (h p c) -> o h p c", h=HPC, p=HEADDIM),
                in_=lam_src)
            lam_flat = sb.tile([128, HPC * HEADDIM * NCHUNK], BF16, tag="bt")
            lam_rd = dsc.tile([1, HPC * HEADDIM * NCHUNK], BF16)
            nc.gpsimd.dma_start(out=lam_rd, in_=lam_row)
            nc.gpsimd.dma_start(
                out=lam_flat,
                in_=bass.AP(tensor=lam_rd.tensor, offset=lam_rd.offset,
                            ap=[[0, 128], [1, HPC * HEADDIM * NCHUNK]]))

            # inter-chunk recurrence (Sg recycles the XT slot)
            Sg = sb.tile([128, HPC * HEADDIM * NCHUNK], BF16, tag="big2")
            nc.vector.tensor_tensor_scan(out=Sg, data0=lam_flat, data1=S_all,
                                         initial=0.0, op0=OP.mult, op1=OP.add)

            # per-position projections: y32s[(hh,p), t, pos] = C32 . Sg[:,(h,p,pos-1)]
            py = psA.tile([128, 2, NPOS], F32, tag="psc")
            nc.vector.memset(py[:, :, 0:1], 0.0)
            first = True
            for pos in range(1, NPOS):
                for t in range(2):
                    lhs = bass.AP(
                        tensor=Sg.tensor,
                        offset=Sg.offset + (2 * t) * HEADDIM * NCHUNK + (pos - 1),
                        ap=[list(Sg.ap[0]), [HEADDIM * NCHUNK, 2], [NCHUNK, 64]])
                    nc.tensor.matmul(py[:, t, pos:pos + 1], lhs,
                                     C32b[:, pos:pos + 1],
                                     start=first, stop=(pos == NPOS - 1 and t == 1),
                                     skip_group_check=True)
                    first = False
            y32 = sb.tile([128, 2, NPOS], F32)
            nc.vector.tensor_mul(y32, py, dA_bc)
            tloc = sb.tile([128, 2, NPOS], F32)
            nc.vector.tensor_mul(tloc, x32, f_bc)
            nc.vector.tensor_add(y32, y32, tloc)
            nc.vector.tensor_mul(y32, y32, zs)
            nc.sync.dma_start(out=y32g_out, in_=y32)

    nc.compile()
    _L1 = nc
    return nc


# ----------------------------------------------------------------------------
# Launch 2 program: gated RMSNorm + out_proj + MLP on the 32 rows
# ----------------------------------------------------------------------------
_L2 = None


def build_l2():
    global _L2
    if _L2 is not None:
        return _L2
    nc = bacc.Bacc("TRN2", target_bir_lowering=False, debug=False,
                   num_devices=NCORES)

    y32g = nc.dram_tensor("y32g_full", (128, 16, NPOS), F32, kind="ExternalInput").ap()
    norm_w = nc.dram_tensor("norm_w", (128, 16), F32, kind="ExternalInput").ap()
    w_out = nc.dram_tensor("w_outT", (16, 128, D_MODEL), BF16, kind="ExternalInput").ap()
    w1 = nc.dram_tensor("w1T", (KT, 128, HIDDEN), BF16, kind="ExternalInput").ap()
    b1 = nc.dram_tensor("b1", (128, HIDDEN // 128), F32, kind="ExternalInput").ap()
    w2 = nc.dram_tensor("w2T", (32, 128, 512), BF16, kind="ExternalInput").ap()
    b2 = nc.dram_tensor("b2", (128, 4), F32, kind="ExternalInput").ap()
    out32 = nc.dram_tensor("out32", (128, 4, NPOS), F32, kind="ExternalOutput").ap()

    with tile.TileContext(nc) as tc:
        import contextlib
        with contextlib.ExitStack() as ctx:
            sb = ctx.enter_context(tc.tile_pool(name="sb", bufs=1))
            psp = ctx.enter_context(tc.tile_pool(name="ps", bufs=1, space="PSUM"))
            dsc = ctx.enter_context(tc.tile_pool(name="dsc", bufs=1, space="DRAM"))

            y_sb = sb.tile([128, 16, NPOS], F32)
            nc.sync.dma_start(out=y_sb, in_=y32g)
            nw_sb = sb.tile([128, 16], F32)
            nc.sync.dma_start(out=nw_sb, in_=norm_w)
            wo_sb = sb.tile([128, 16, D_MODEL], BF16)
            nc.sync.dma_start(out=wo_sb, in_=w_out.rearrange("k p c -> p k c"))
            w1_sb = sb.tile([128, KT, HIDDEN], BF16)
            nc.sync.dma_start(out=w1_sb, in_=w1.rearrange("k p c -> p k c"))
            b1_sb = sb.tile([128, HIDDEN // 128], F32)
            nc.sync.dma_start(out=b1_sb, in_=b1)
            w2_sb = sb.tile([128, 32, 512], BF16)
            nc.sync.dma_start(out=w2_sb, in_=w2.rearrange("k p c -> p k c"))
            b2_sb = sb.tile([128, 4], F32)
            nc.sync.dma_start(out=b2_sb, in_=b2)

            # sum of squares over channels (partition x 16 ktiles)
            sq = sb.tile([128, 16, NPOS], F32)
            nc.vector.tensor_mul(sq, y_sb, y_sb)
            ones1 = sb.tile([128, 1], F32)
            nc.vector.memset(ones1, 1.0)
            pss = psp.tile([1, NPOS], F32, tag="pss")
            for k in range(16):
                nc.tensor.matmul(pss, ones1, sq[:, k, :],
                                 start=(k == 0), stop=(k == 15))
            # r = 1/sqrt(mean + eps)
            eps_t = sb.tile([1, 1], F32)
            nc.vector.memset(eps_t, 1e-5)
            rs = sb.tile([1, NPOS], F32)
            nc.scalar.activation(out=rs, in_=pss, func=AF.Sqrt,
                                 bias=eps_t[:, 0:1], scale=1.0 / D_INNER)
            nc.vector.reciprocal(rs, rs)
            r_d = dsc.tile([1, NPOS], F32)
            nc.sync.dma_start(out=r_d, in_=rs)
            r_bc = sb.tile([128, NPOS], F32)
            nc.sync.dma_start(out=r_bc,
                              in_=bass.AP(tensor=r_d.tensor, offset=r_d.offset,
                                          ap=[[0, 128], [1, NPOS]]))
            yn = sb.tile([128, 16, NPOS], BF16)
            for k in range(16):
                nc.vector.scalar_tensor_tensor(out=yn[:, k, :], in0=y_sb[:, k, :],
                                               scalar=nw_sb[:, k:k + 1], in1=r_bc,
                                               op0=OP.mult, op1=OP.mult)
            # h = w_outT.T @ yn   [1024, 32]
            h_sb = sb.tile([128, 8, NPOS], BF16)
            for mt in range(8):
                ph = psp.tile([128, NPOS], F32, tag="ph", bufs=2)
                for k in range(16):
                    nc.tensor.matmul(ph, wo_sb[:, k, mt * 128:(mt + 1) * 128],
                                     yn[:, k, :], start=(k == 0), stop=(k == 15))
                nc.vector.tensor_copy(out=h_sb[:, mt, :], in_=ph)
            # g = gelu(w1T.T @ h + b1)  [4096, 32]
            g_sb = sb.tile([128, 32, NPOS], BF16)
            for mt in range(32):
                pg = psp.tile([128, NPOS], F32, tag="pg", bufs=2)
                for k in range(KT):
                    nc.tensor.matmul(pg, w1_sb[:, k, mt * 128:(mt + 1) * 128],
                                     h_sb[:, k, :], start=(k == 0), stop=(k == KT - 1))
                nc.scalar.activation(out=g_sb[:, mt, :], in_=pg, func=AF.Gelu,
                                     bias=b1_sb[:, mt:mt + 1], scale=1.0)
            # out = w2T.T @ g + b2   [512, 32] per core
            for mt in range(4):
                po = psp.tile([128, NPOS], F32, tag="po", bufs=2)
                for k in range(32):
                    nc.tensor.matmul(po, w2_sb[:, k, mt * 128:(mt + 1) * 128],
                                     g_sb[:, k, :], start=(k == 0), stop=(k == 31))
                ot = sb.tile([128, NPOS], F32, tag="ot", bufs=2)
                nc.vector.tensor_scalar(ot, po, b2_sb[:, mt:mt + 1], None, OP.add)
                nc.sync.dma_start(out=out32[:, mt, :], in_=ot)

    nc.compile()
    _L2 = nc
    return nc


# ----------------------------------------------------------------------------
# Host-side prep + glue
# ----------------------------------------------------------------------------

def _prep_l1_maps(inputs):
    x = _f32(inputs["x"]).reshape(L, D_MODEL)
    xT = np.ascontiguousarray(x.T)                       # [1024, 8192]
    xT_b = _bf(xT)
    pos = np.arange(NPOS) * POS_STRIDE
    xTpos = _bf(xT[:, pos])
    # window tokens (pos, d): t*-3+d, zero-padded below 0
    win_idx = (pos[:, None] + np.arange(D_CONV)[None, :] - (D_CONV - 1)).reshape(-1)
    xTwin = np.zeros((D_MODEL, NPOS * D_CONV), np.float32)
    valid = win_idx >= 0
    xTwin[:, valid] = xT[:, win_idx[valid]]
    xTwin = _bf(xTwin)

    w_all = _f32(inputs["in_proj_w"])                    # [4384, 1024]
    conv_w = _f32(inputs["conv_w"])                      # [2304, 4]
    conv_b = _f32(inputs["conv_b"])                      # [2304]
    dt_bias = _f32(inputs["dt_bias"])                    # [32]
    A = -np.exp(_f32(inputs["A_log"]))                   # [32]
    Dp = _f32(inputs["D"])                               # [32]

    w_cT = _bf(w_all[D_INNER + D_INNER + D_STATE:
                     D_INNER + D_INNER + 2 * D_STATE].T.reshape(KT, 128, 128))
    cw_c = _f32(conv_w[D_INNER + D_STATE:])              # [128, 4] C channels
    conv_b_c = _f32(conv_b[D_INNER + D_STATE:]).reshape(128, 1)

    maps = []
    for k in range(NCORES):
        xs = 256 * k
        cols = np.concatenate([
            np.arange(D_INNER + xs, D_INNER + xs + 256),          # x slice
            np.arange(2 * D_INNER, 2 * D_INNER + D_STATE),        # B
        ])
        w_in = _bf(w_all[cols].T.reshape(KT, 128, NCOL))
        dt_cols = np.arange(D_IN_PROJ - NHEADS + HPC * k,
                            D_IN_PROJ - NHEADS + HPC * k + HPC)
        w_dt = _bf(w_all[dt_cols].T.reshape(KT, 128, HPC))
        w_z = _bf(w_all[xs:xs + 256].T.reshape(KT, 128, 256))
        # conv channels for this core: x slice (256) + B (128)
        ch_x = np.arange(xs, xs + 256)
        ch_B = np.arange(D_INNER, D_INNER + D_STATE)
        dw = np.zeros((3, D_CONV, 128, 128), np.float32)
        cb = np.zeros((128, 3), np.float32)
        for cht, chs in enumerate([ch_x[:128], ch_x[128:], ch_B]):
            for j in range(D_CONV):
                dw[cht, j] = np.diag(conv_w[chs, j])
            cb[:, cht] = conv_b[chs]
        heads = np.arange(HPC * k, HPC * k + HPC)
        maps.append({
            "xT": xT_b, "xTpos": xTpos, "xTwin": xTwin,
            "w_in": w_in, "w_dt": w_dt, "w_c": w_cT, "w_z": w_z,
            "diag_w": _bf(dw), "cw_c": cw_c, "conv_b": cb,
            "conv_b_c": conv_b_c,
            "dtb4": dt_bias[heads].reshape(HPC, 1).astype(np.float32),
            "A4": A[heads].reshape(HPC, 1).astype(np.float32),
            "D4": Dp[heads].reshape(HPC, 1).astype(np.float32),
        })
    return maps


def _prep_l2_maps(inputs, y32g_full):
    # ch = kt*128 + p -> norm_w_sb[p, kt] = norm_w[kt*128+p]
    nw = _f32(inputs["norm_w"]).reshape(16, 128).transpose(1, 0).copy()
    w_out = _f32(inputs["mamba_out_w"])                  # [1024, 2048]
    w_outT = _bf(w_out.T.reshape(16, 128, D_MODEL))
    w1 = _f32(inputs["mlp_w1"])                          # [4096, 1024]
    w1T = _bf(w1.T.reshape(KT, 128, HIDDEN))
    b1 = _f32(inputs["mlp_b1"]).reshape(32, 128).transpose(1, 0).copy()
    w2 = _f32(inputs["mlp_w2"])                          # [4096, 4096]
    maps = []
    for k in range(NCORES):
        cols = slice(512 * k, 512 * k + 512)
        w2T = _bf(w2[cols].T.reshape(32, 128, 512))
        b2 = _f32(inputs["mlp_b2"])[cols].reshape(4, 128).transpose(1, 0).copy()
        maps.append({
            "y32g_full": y32g_full, "norm_w": nw, "w_outT": w_outT,
            "w1T": w1T, "b1": b1, "w2T": w2T, "b2": b2,
        })
    return maps


LAST_RESULTS = []


def kernel(**inputs) -> np.ndarray:
    trace = os.environ.get("KERNEL_TRACE", "0") == "1"
    LAST_RESULTS.clear()
    nc1 = build_l1()
    maps1 = _prep_l1_maps(inputs)
    res1 = bass_utils.run_bass_kernel_spmd(nc1, maps1, core_ids=list(range(NCORES)),
                                           trace=trace)
    LAST_RESULTS.append(res1)
    # assemble y32g_full [128, 16, 32]: ch = 256*k + t*128 + p -> kt = 2k+t
    y32g_full = np.zeros((128, 16, NPOS), np.float32)
    for k in range(NCORES):
        y = res1.results[k]["y32g"]                      # [128, 2, 32]
        y32g_full[:, 2 * k:2 * k + 2, :] = y
    nc2 = build_l2()
    maps2 = _prep_l2_maps(inputs, y32g_full)
    res2 = bass_utils.run_bass_kernel_spmd(nc2, maps2, core_ids=list(range(NCORES)),
                                           trace=trace)
    LAST_RESULTS.append(res2)
    out = np.zeros((NPOS, HIDDEN), np.float32)
    for k in range(NCORES):
        o = res2.results[k]["out32"]                     # [128, 4, 32]
        # out[pos, 512k + mt*128 + p] = o[p, mt, pos]
        out[:, 512 * k:512 * (k + 1)] = o.transpose(2, 1, 0).reshape(NPOS, 512)
    return out.astype(np.float32)



# revision 14
# speedup vs baseline: 1.0141x; 1.0141x over previous
"""Trainium2 Bass kernel for nn_Connection_75411035783724 (Mamba2 block + MLP head).

Single fused launch, tensor-parallel over the 32 Mamba2 heads across 8 cores
(4 heads each).  Per core: in_proj column-slice (x-channels + B + dt), causal
depthwise conv as accumulating diagonal matmuls, chunked-SSD scan (chunk 256),
then an on-device AllGather of the 32 gated position outputs feeds the
RMSNorm + out_proj + MLP tail (MLP2 column-sharded) in the same NEFF.

Schedule notes (why the emission order looks scrambled):
- conv matmuls for group g-1 are emitted after in_proj for group g so the PE
  never waits on same-group PSUM evictions.
- the decay pipe runs per 1024-token batch entirely in the natural_log_exp
  activation-table set (softplus = ln(1+exp)); conv-silu is the only other
  scalar table in the loop.
- all partition-broadcasts ([4]->[128] etc.) are one-hot matmuls, not DRAM
  round-trips; per-chunk cumsums run on gpsimd; the inter-chunk scan is split
  across vector+gpsimd at the tail.
- MLP weights stream through small SBUF rings so their DMA overlaps PE.
"""
import os
import sys
import numpy as np
import ml_dtypes

sys.path.insert(0, "/opt/trn_rl_repo")

import concourse.bass as bass
import concourse.tile as tile
import concourse.mybir as mybir
from concourse import bacc
from concourse import bass_utils

F32 = mybir.dt.float32
BF16 = mybir.dt.bfloat16
AF = mybir.ActivationFunctionType
OP = mybir.AluOpType
BF = ml_dtypes.bfloat16

# Model dims
D_MODEL = 1024
HIDDEN = 4096
D_STATE = 128       # n
D_CONV = 4
D_INNER = 2048
HEADDIM = 64        # p
NHEADS = 32
CONV_DIM = D_INNER + 2 * D_STATE            # 2304
D_IN_PROJ = 2 * D_INNER + 2 * D_STATE + NHEADS  # 4384
L = 8192            # tokens
NPOS = 32           # output positions (first token of each frame)
POS_STRIDE = 256
NCORES = 8
HPC = 4             # heads per core
Q = 256             # chunk length
NCHUNK = L // Q     # 32
KT = D_MODEL // 128  # 8 K-tiles
NG = 16             # token groups of 512
GSZ = 512
BSZ = 2 * GSZ       # 1024-token batches for the decay pipe
NB = L // BSZ       # 8
CPB = BSZ // Q      # 4 chunks per batch
# in_proj col slice per core: [x 256 | B 128]; dt handled via its own tensor
NCOL = 256 + 128  # 384
MT_SPEC = [(0, 128), (128, 128), (256, 128)]  # (col0, width)
SFLAT = HPC * HEADDIM * NCHUNK  # 8192


def _bf(x):
    return np.ascontiguousarray(np.asarray(x, dtype=np.float32)).astype(BF)


def _f32(x):
    return np.ascontiguousarray(np.asarray(x, dtype=np.float32))


_NC = None


def build():
    global _NC
    if _NC is not None:
        return _NC
    nc = bacc.Bacc("TRN2", target_bir_lowering=False, debug=False,
                   num_devices=NCORES)

    def din(name, shape, dt):
        return nc.dram_tensor(name, shape, dt, kind="ExternalInput").ap()

    xT = din("xT", (D_MODEL, L), BF16)
    xTpos = din("xTpos", (D_MODEL, NPOS), BF16)
    xTwin = din("xTwin", (D_MODEL, NPOS * D_CONV), BF16)
    w_in = din("w_in", (KT, 128, NCOL), BF16)
    w_dt = din("w_dt", (KT, 128, HPC), BF16)
    w_c = din("w_c", (KT, 128, 128), BF16)
    w_z = din("w_z", (KT, 128, 256), BF16)
    diag_w = din("diag_w", (3, D_CONV, 128, 128), BF16)
    cw_c = din("cw_c", (128, D_CONV), F32)
    conv_b = din("conv_b", (128, 3), F32)
    conv_b_c = din("conv_b_c", (128, 1), F32)
    dtb4 = din("dtb4", (HPC, 1), F32)
    A4 = din("A4", (HPC, 1), F32)
    D4 = din("D4", (HPC, 1), F32)
    oh_w = din("oh_w", (HPC, 256), BF16)        # one-hot head->partition map
    # MLP tail
    norm_w = din("norm_w", (128, 16), F32)
    w_out = din("w_outT", (16, 128, D_MODEL), BF16)
    w1 = din("w1T", (KT, 128, HIDDEN), BF16)
    b1 = din("b1", (128, HIDDEN // 128), F32)
    w2 = din("w2T", (32, 128, 512), BF16)
    b2 = din("b2", (128, 4), F32)
    out32 = nc.dram_tensor("out32", (128, 4, NPOS), F32,
                           kind="ExternalOutput").ap()

    with tile.TileContext(nc) as tc:
        import contextlib
        with contextlib.ExitStack() as ctx:
            sb = ctx.enter_context(tc.tile_pool(name="sb", bufs=1))
            ring = ctx.enter_context(tc.tile_pool(name="ring", bufs=1))
            dsc = ctx.enter_context(tc.tile_pool(name="dsc", bufs=1, space="DRAM"))
            psA = ctx.enter_context(tc.tile_pool(name="psA", bufs=1, space="PSUM"))

            # ---- resident weights/constants
            w_in_sb = sb.tile([128, KT, NCOL], BF16)
            nc.sync.dma_start(out=w_in_sb, in_=w_in.rearrange("k p c -> p k c"))
            w_dt_sb = sb.tile([128, KT, HPC], BF16)
            nc.sync.dma_start(out=w_dt_sb, in_=w_dt.rearrange("k p c -> p k c"))
            w_c_sb = sb.tile([128, KT, 128], BF16)
            nc.sync.dma_start(out=w_c_sb, in_=w_c.rearrange("k p c -> p k c"))
            w_z_sb = sb.tile([128, KT, 256], BF16)
            nc.sync.dma_start(out=w_z_sb, in_=w_z.rearrange("k p c -> p k c"))
            diag_sb = sb.tile([128, 3, D_CONV, 128], BF16)
            nc.sync.dma_start(out=diag_sb, in_=diag_w.rearrange("c j a b -> a c j b"))
            cw_sb = sb.tile([128, D_CONV], F32)
            nc.sync.dma_start(out=cw_sb, in_=cw_c)
            cb_sb = sb.tile([128, 3], F32)
            nc.sync.dma_start(out=cb_sb, in_=conv_b)
            cbc_sb = sb.tile([128, 1], F32)
            nc.sync.dma_start(out=cbc_sb, in_=conv_b_c)
            dtb_sb = sb.tile([HPC, 1], F32)
            nc.sync.dma_start(out=dtb_sb, in_=dtb4)
            A_sb = sb.tile([HPC, 1], F32)
            nc.sync.dma_start(out=A_sb, in_=A4)
            D_sb = sb.tile([HPC, 1], F32)
            nc.sync.dma_start(out=D_sb, in_=D4)
            oh_sb = sb.tile([HPC, 256], BF16)
            nc.sync.dma_start(out=oh_sb, in_=oh_w)
            xtp_sb = sb.tile([128, KT, NPOS], BF16)
            nc.sync.dma_start(out=xtp_sb, in_=xTpos.rearrange("(k p) t -> p k t", p=128))
            xtw_sb = sb.tile([128, KT, NPOS * D_CONV], BF16)
            nc.sync.dma_start(out=xtw_sb, in_=xTwin.rearrange("(k p) t -> p k t", p=128))
            nw_sb = sb.tile([128, 16], F32)
            nc.sync.dma_start(out=nw_sb, in_=norm_w)
            b1_sb = sb.tile([128, HIDDEN // 128], F32)
            nc.sync.dma_start(out=b1_sb, in_=b1)
            b2_sb = sb.tile([128, 4], F32)
            nc.sync.dma_start(out=b2_sb, in_=b2)

            # ---- persistent big buffers
            xbcc = sb.tile([128, 3, L], BF16, tag="big1")  # conv+silu [x0|x1|B]
            XT = sb.tile([128, L // 128, 256], BF16, tag="big2")  # transposed w*x
            BT = sb.tile([128, L // 128, 128], BF16, tag="bt")    # transposed B
            S_all = sb.tile([128, SFLAT], BF16)      # per-chunk states (h,p,c)
            lamA = sb.tile([HPC, NCHUNK], F32)       # per-chunk decay
            dAA = sb.tile([HPC, NCHUNK], F32)        # exp(a) at chunk starts
            dt_pos = sb.tile([HPC, NCHUNK], F32)     # dt at chunk starts
            x32 = sb.tile([128, 2, NPOS], F32)
            B32 = sb.tile([128, NPOS], F32)
            ones4 = sb.tile([HPC, BSZ], F32)
            one4 = sb.tile([HPC, 1], F32)
            dt2_slots = [sb.tile([HPC, BSZ], F32, tag="dt2a", name="dt2a"),
                         sb.tile([HPC, BSZ], F32, tag="dt2b", name="dt2b")]
            a2_s = sb.tile([HPC, BSZ], F32)
            s2_s = sb.tile([HPC, BSZ], F32)
            w2b_s = sb.tile([HPC, BSZ], BF16)
            nc.vector.memset(ones4, 1.0)
            nc.vector.memset(one4, 1.0)
            # zero at chunk starts -> cumsum resets there
            z_ap = bass.AP(tensor=ones4.tensor, offset=ones4.offset,
                           ap=[list(ones4.ap[0]), [Q, CPB]])
            nc.vector.memset(z_ap, 0.0)

            # ================= pre-loop tail-independent work ================
            # C at positions (conv window matmul + 4-tap conv + silu)
            C32 = sb.tile([128, NPOS], F32)
            pcw = psA.tile([128, NPOS * D_CONV], F32, tag="pin0")
            for k in range(KT):
                nc.tensor.matmul(pcw, w_c_sb[:, k, :], xtw_sb[:, k, :],
                                 start=(k == 0), stop=(k == KT - 1))
            tmpc = sb.tile([128, NPOS], F32)
            for j in range(D_CONV):
                src = bass.AP(tensor=pcw.tensor, offset=pcw.offset + j,
                              ap=[list(pcw.ap[0]), [D_CONV, NPOS]])
                if j == 0:
                    nc.vector.tensor_scalar_mul(tmpc, src, cw_sb[:, 0:1])
                else:
                    nc.vector.scalar_tensor_tensor(
                        out=tmpc, in0=src, scalar=cw_sb[:, j:j + 1], in1=tmpc,
                        op0=OP.mult, op1=OP.add)
            nc.scalar.activation(out=C32, in_=tmpc, func=AF.Silu,
                                 bias=cbc_sb[:, 0:1], scale=1.0)
            C32b = sb.tile([128, NPOS], BF16)
            nc.vector.tensor_copy(out=C32b, in_=C32)
            # z gate values at positions
            pz = psA.tile([128, 2, NPOS], F32, tag="pin1")
            for t in range(2):
                for k in range(KT):
                    nc.tensor.matmul(pz[:, t, :],
                                     w_z_sb[:, k, t * 128:(t + 1) * 128],
                                     xtp_sb[:, k, :],
                                     start=(k == 0), stop=(k == KT - 1))
            zs = sb.tile([128, 2, NPOS], F32)
            nc.scalar.activation(out=zs, in_=pz, func=AF.Silu)

            # ================= main fused loop =================
            xt_pool = ring
            xt_tiles = {}

            def load_xt(g):
                t = xt_pool.tile([128, KT, GSZ], BF16, tag="xt", bufs=2)
                sl = slice(g * GSZ, (g + 1) * GSZ)
                nc.sync.dma_start(
                    out=t, in_=xT.rearrange("(k p) t -> p k t", p=128)[:, :, sl])
                xt_tiles[g] = t

            load_xt(0)
            load_xt(1)

            pins = {}
            pdts = {}
            xbc_tiles = {}

            def emit_inproj(g):
                xt_g = xt_tiles.pop(g)
                ps = []
                for mt, (c0, cw) in enumerate(MT_SPEC):
                    p = psA.tile([cw, GSZ], F32, tag=f"pin{mt}")
                    for k in range(KT):
                        nc.tensor.matmul(p, w_in_sb[:, k, c0:c0 + cw],
                                         xt_g[:, k, :],
                                         start=(k == 0), stop=(k == KT - 1))
                    ps.append(p)
                pdt = psA.tile([HPC, GSZ], F32, tag="pdt", bufs=2)
                for k in range(KT):
                    nc.tensor.matmul(pdt, w_dt_sb[:, k, :], xt_g[:, k, :],
                                     start=(k == 0), stop=(k == KT - 1))
                pins[g] = ps
                pdts[g] = pdt

            def emit_evict(g):
                ps = pins.pop(g)
                xbc_g = ring.tile([128, 3, GSZ + 3], BF16, tag="xbc", bufs=2)
                if g == 0:
                    nc.vector.memset(xbc_g[:, :, 0:3], 0.0)
                else:
                    prev = xbc_tiles[g - 1]
                    nc.vector.tensor_copy(out=xbc_g[:, :, 0:3],
                                          in_=prev[:, :, GSZ:GSZ + 3])
                for cht in range(3):
                    if cht != 2:
                        nc.vector.tensor_copy(out=xbc_g[:, cht, 3:], in_=ps[cht])
                    else:
                        nc.scalar.copy(out=xbc_g[:, cht, 3:], in_=ps[cht])
                xbc_tiles[g] = xbc_g
                # dt PSUM -> SBUF slot
                b, half = divmod(g, 2)
                dt2 = dt2_slots[b % 2]
                pdt = pdts.pop(g)
                nc.vector.tensor_copy(out=dt2[:, half * GSZ:(half + 1) * GSZ],
                                      in_=pdt)

            def emit_conv(g):
                xbc_g = xbc_tiles[g]
                sl = slice(g * GSZ, (g + 1) * GSZ)
                for cht in range(3):
                    pc = psA.tile([128, GSZ], F32, tag="psh", bufs=3)
                    for j in range(D_CONV):
                        nc.tensor.matmul(pc, diag_sb[:, cht, j, :],
                                         xbc_g[:, cht, j:j + GSZ],
                                         start=(j == 0), stop=(j == D_CONV - 1))
                    nc.scalar.activation(out=xbcc[:, cht, sl], in_=pc,
                                         func=AF.Silu,
                                         bias=cb_sb[:, cht:cht + 1], scale=1.0)
                if g >= 2:
                    del xbc_tiles[g - 1]

            def emit_decay(b):
                # all scalar ops below live in the natural_log_exp table set
                dt2 = dt2_slots[b % 2]
                # softplus: dt = ln(1 + exp(v + bias))
                nc.scalar.activation(out=a2_s, in_=dt2, func=AF.Exp,
                                     bias=dtb_sb[:, 0:1], scale=1.0)
                nc.scalar.activation(out=dt2, in_=a2_s, func=AF.Ln,
                                     bias=one4[:, 0:1], scale=1.0)
                # a = dt*A ; per-chunk inclusive cumsum (reset via zeroed mask)
                a2 = a2_s
                nc.vector.tensor_scalar_mul(a2, dt2, A_sb[:, 0:1])
                s2 = s2_s
                nc.vector.tensor_tensor_scan(out=s2, data0=ones4, data1=a2,
                                             initial=0.0, op0=OP.mult, op1=OP.add)
                # extracts at chunk starts / ends
                cpos = b * CPB
                src = bass.AP(tensor=a2.tensor, offset=a2.offset,
                              ap=[list(a2.ap[0]), [Q, CPB]])
                nc.scalar.activation(out=dAA[:, cpos:cpos + CPB], in_=src,
                                     func=AF.Exp)
                src = bass.AP(tensor=dt2.tensor, offset=dt2.offset,
                              ap=[list(dt2.ap[0]), [Q, CPB]])
                nc.vector.tensor_copy(out=dt_pos[:, cpos:cpos + CPB], in_=src)
                src = bass.AP(tensor=s2.tensor, offset=s2.offset + Q - 1,
                              ap=[list(s2.ap[0]), [Q, CPB]])
                nc.scalar.activation(out=lamA[:, cpos:cpos + CPB], in_=src,
                                     func=AF.Exp)
                # w = exp(stot - s) * dt
                for cc in range(CPB):
                    stot = bass.AP(tensor=s2.tensor,
                                   offset=s2.offset + cc * Q + Q - 1,
                                   ap=[list(s2.ap[0]), [1, 1]])
                    nc.vector.tensor_scalar(s2[:, cc * Q:(cc + 1) * Q],
                                            s2[:, cc * Q:(cc + 1) * Q],
                                            stot, None, OP.subtract)
                nc.scalar.activation(out=s2, in_=s2, func=AF.Exp, scale=-1.0)
                nc.vector.tensor_mul(w2b_s, s2, dt2)
                return w2b_s

            def emit_scale(b, w2b):
                # broadcast w [4,tok] -> [128,tok] via one-hot matmul, scale x,
                # transpose into XT; BT transpose for this batch's B channels
                bsl = slice(b * BSZ, (b + 1) * BSZ)
                for half in range(2):
                    hsl = slice(half * GSZ, (half + 1) * GSZ)
                    for t in range(2):
                        pw = psA.tile([128, GSZ], F32, tag="psh", bufs=3)
                        nc.tensor.matmul(pw, oh_sb[:, t * 128:(t + 1) * 128],
                                         w2b[:, hsl], start=True, stop=True)
                        xs = ring.tile([128, GSZ], BF16, tag="xs", bufs=2)
                        nc.vector.tensor_mul(
                            xs, xbcc[:, t, b * BSZ + half * GSZ:
                                     b * BSZ + (half + 1) * GSZ], pw)
                        eng = nc.sync if t == 0 else nc.scalar
                        eng.dma_start_transpose(
                            out=XT[:, 8 * b + 4 * half:8 * b + 4 * (half + 1),
                                   t * 128:(t + 1) * 128],
                            in_=xs)
                nc.sync.dma_start_transpose(out=BT[:, 8 * b:8 * (b + 1), :],
                                            in_=xbcc[:, 2, bsl])

            def emit_scan(b):
                for cc in range(CPB):
                    c = b * CPB + cc
                    pc2 = psA.tile([128, HPC * HEADDIM], F32, tag="psh", bufs=3)
                    for k2 in range(2):
                        T = 2 * c + k2
                        nc.tensor.matmul(pc2, BT[:, T, :], XT[:, T, :],
                                         start=(k2 == 0), stop=(k2 == 1))
                    dst = bass.AP(tensor=S_all.tensor, offset=S_all.offset + c,
                                  ap=[list(S_all.ap[0]), [NCHUNK, HPC * HEADDIM]])
                    if c % 2 == 0:
                        nc.vector.tensor_copy(
                            out=dst, in_=pc2)
                    else:
                        nc.scalar.copy(out=dst, in_=pc2)

            w2b_live = {}
            for g in range(NG + 2):
                if g + 2 < NG:
                    load_xt(g + 2)
                if g < NG:
                    emit_inproj(g)
                    emit_evict(g)
                if g >= 1 and g - 1 < NG:
                    emit_conv(g - 1)
                if g >= 2 and g % 2 == 0:
                    b = (g - 2) // 2
                    w2b_live[b] = emit_decay(b)
                    emit_scale(b, w2b_live[b])
                if g >= 3 and g % 2 == 1:
                    emit_scan((g - 3) // 2)

            # ================= tail =================
            # batched position extracts from xbcc
            for cht in range(2):
                s_ap = bass.AP(tensor=xbcc.tensor, offset=xbcc.offset + cht * L,
                               ap=[list(xbcc.ap[0]), [POS_STRIDE, NPOS]])
                nc.vector.tensor_copy(out=x32[:, cht, :], in_=s_ap)
            s_ap = bass.AP(tensor=xbcc.tensor, offset=xbcc.offset + 2 * L,
                           ap=[list(xbcc.ap[0]), [POS_STRIDE, NPOS]])
            nc.vector.tensor_copy(out=B32, in_=s_ap)

            # BC row = sum_n B32*C32 -> [1, NPOS] -> [HPC, NPOS] via one-hot
            bc_t = sb.tile([128, NPOS], F32)
            nc.vector.tensor_mul(bc_t, B32, C32)
            bc_tb = sb.tile([128, NPOS], BF16)
            nc.vector.tensor_copy(out=bc_tb, in_=bc_t)
            ones128b = sb.tile([1, 128], BF16)
            nc.vector.memset(ones128b, 1.0)
            oh14 = sb.tile([1, HPC], BF16)
            nc.vector.memset(oh14, 1.0)
            pbc = psA.tile([1, NPOS], F32, tag="pdt", bufs=2)
            onesc = sb.tile([128, 1], BF16)
            nc.vector.memset(onesc, 1.0)
            nc.tensor.matmul(pbc, onesc, bc_tb, start=True, stop=True)
            bc_row = sb.tile([1, NPOS], BF16)
            nc.vector.tensor_copy(out=bc_row, in_=pbc)
            pbc4 = psA.tile([HPC, NPOS], F32, tag="pdt", bufs=2)
            nc.tensor.matmul(pbc4, oh14, bc_row, start=True, stop=True)
            f4 = sb.tile([HPC, NPOS], F32)
            nc.vector.tensor_mul(f4, dt_pos, pbc4)
            nc.vector.tensor_scalar(f4, f4, D_sb[:, 0:1], None, OP.add)
            f4b = sb.tile([HPC, NPOS], BF16)
            nc.vector.tensor_copy(out=f4b, in_=f4)
            dAAb = sb.tile([HPC, NCHUNK], BF16)
            nc.vector.tensor_copy(out=dAAb, in_=dAA)

            # dA/f broadcasts [4,32] -> [128,2,32] via one-hot matmuls
            pda = psA.tile([128, 2, NPOS], F32, tag="pin0")
            pf = psA.tile([128, 2, NPOS], F32, tag="pin1")
            for t in range(2):
                nc.tensor.matmul(pda[:, t, :], oh_sb[:, t * 128:(t + 1) * 128],
                                 dAAb, start=True, stop=True)
                nc.tensor.matmul(pf[:, t, :], oh_sb[:, t * 128:(t + 1) * 128],
                                 f4b, start=True, stop=True)
            dA_bc = sb.tile([128, 2, NPOS], F32)
            nc.vector.tensor_copy(out=dA_bc, in_=pda)
            f_bc = sb.tile([128, 2, NPOS], F32)
            nc.vector.tensor_copy(out=f_bc, in_=pf)

            # lambda -> lam_flat [128, (h,p,c)] via DRAM stride-0 expansion
            nc.vector.memset(lamA[:, 0:1], 0.0)
            lam16 = sb.tile([HPC, NCHUNK], BF16)
            nc.vector.tensor_copy(out=lam16, in_=lamA)
            lam_d = dsc.tile([HPC, NCHUNK], BF16)
            nc.gpsimd.dma_start(out=lam_d, in_=lam16)
            lam_row = sb.tile([1, SFLAT], BF16)
            lam_src = bass.AP(tensor=lam_d.tensor, offset=lam_d.offset,
                              ap=[[0, 1], [NCHUNK, HPC], [0, HEADDIM],
                                  [1, NCHUNK]])
            nc.gpsimd.dma_start(
                out=lam_row.rearrange("o (h p c) -> o h p c", h=HPC, p=HEADDIM),
                in_=lam_src)
            lam_flat = sb.tile([128, SFLAT], BF16, tag="bt")
            lam_rd = dsc.tile([1, SFLAT], BF16)
            nc.gpsimd.dma_start(out=lam_rd, in_=lam_row)
            nc.gpsimd.dma_start(
                out=lam_flat,
                in_=bass.AP(tensor=lam_rd.tensor, offset=lam_rd.offset,
                            ap=[[0, 128], [1, SFLAT]]))

            # inter-chunk recurrence
            Sg = sb.tile([128, SFLAT], BF16, tag="big2")
            nc.vector.tensor_tensor_scan(
                out=Sg, data0=lam_flat,
                data1=S_all, initial=0.0, op0=OP.mult, op1=OP.add)

            # per-position projections: py[(hh,p), t, pos] = C32 . Sg[:,(h,p,pos-1)]
            py = psA.tile([128, 2, NPOS], F32, tag="pin2")
            nc.vector.memset(py[:, :, 0:1], 0.0)
            first = True
            for pos in range(1, NPOS):
                for t in range(2):
                    lhs = bass.AP(
                        tensor=Sg.tensor,
                        offset=Sg.offset + (2 * t) * HEADDIM * NCHUNK + (pos - 1),
                        ap=[list(Sg.ap[0]), [HEADDIM * NCHUNK, 2], [NCHUNK, 64]])
                    nc.tensor.matmul(py[:, t, pos:pos + 1], lhs,
                                     C32b[:, pos:pos + 1],
                                     start=first, stop=(pos == NPOS - 1 and t == 1),
                                     skip_group_check=True)
                    first = False
            y32 = sb.tile([128, 2, NPOS], F32)
            nc.vector.tensor_mul(y32, py, dA_bc)
            tloc = sb.tile([128, 2, NPOS], F32)
            nc.vector.tensor_mul(tloc, x32, f_bc)
            nc.vector.tensor_add(y32, y32, tloc)
            nc.vector.tensor_mul(y32, y32, zs)

            # ---- MLP weight prefetch helpers (stream through small rings)
            wo_tiles = [None] * 8

            def load_wo(mt):
                t = ring.tile([128, 16, 128], BF16, tag="xt", bufs=2)
                nc.sync.dma_start(
                    out=t,
                    in_=w_out.rearrange("k p c -> p k c")[:, :,
                                                          mt * 128:(mt + 1) * 128])
                wo_tiles[mt] = t

            w1_tiles = [None] * 32

            def load_w1(mt):
                t = ring.tile([128, KT, 128], BF16, tag="w1r", bufs=2)
                nc.sync.dma_start(
                    out=t,
                    in_=w1.rearrange("k p c -> p k c")[:, :,
                                                       mt * 128:(mt + 1) * 128])
                w1_tiles[mt] = t

            w2_tiles = [None] * 4

            def load_w2(mt):
                t = ring.tile([128, 32, 128], BF16, tag="xt", bufs=2)
                nc.sync.dma_start(
                    out=t,
                    in_=w2.rearrange("k p c -> p k c")[:, :,
                                                       mt * 128:(mt + 1) * 128])
                w2_tiles[mt] = t

            # issue what the rings allow now, so weight DMA overlaps the gather
            load_wo(0)
            load_wo(1)
            load_w1(0)
            load_w1(1)

            # ---- AllGather y32 across the 8 cores
            y_bounce = dsc.tile([128, 2, NPOS], F32)
            nc.gpsimd.dma_start(out=y_bounce, in_=y32)
            y_gath = dsc.tile([NCORES, 128, 2, NPOS], F32)
            nc.gpsimd.collective_compute(
                "AllGather", mybir.AluOpType.bypass,
                replica_groups=[list(range(NCORES))],
                ins=[y_bounce.opt()], outs=[y_gath.opt()],
            )
            y_sb = sb.tile([128, 16, NPOS], F32)
            y_src = bass.AP(tensor=y_gath.tensor, offset=y_gath.offset,
                            ap=[[2 * NPOS, 128], [128 * 2 * NPOS, NCORES],
                                [NPOS, 2], [1, NPOS]])
            nc.sync.dma_start(
                out=y_sb.rearrange("p (k t) c -> p k t c", k=NCORES),
                in_=y_src)

            # ---- RMSNorm over all 2048 channels
            sq = sb.tile([128, 16, NPOS], BF16)
            nc.vector.tensor_mul(sq, y_sb, y_sb)
            pss = psA.tile([1, NPOS], F32, tag="pdt", bufs=2)
            for k in range(16):
                nc.tensor.matmul(pss, onesc, sq[:, k, :],
                                 start=(k == 0), stop=(k == 15))
            # r = 1/sqrt(mean + eps) = exp(-0.5*ln(mean + eps)) (stays in ln/exp set)
            eps_t = sb.tile([1, 1], F32)
            nc.vector.memset(eps_t, 1e-5)
            rs = sb.tile([1, NPOS], F32)
            nc.scalar.activation(out=rs, in_=pss, func=AF.Ln,
                                 bias=eps_t[:, 0:1], scale=1.0 / D_INNER)
            rsb = sb.tile([1, NPOS], BF16)
            nc.scalar.activation(out=rsb, in_=rs, func=AF.Exp, scale=-0.5)
            prs = psA.tile([128, NPOS], F32, tag="pdt", bufs=2)
            nc.tensor.matmul(prs, ones128b, rsb, start=True, stop=True)
            r_bc = sb.tile([128, NPOS], F32)
            nc.vector.tensor_copy(out=r_bc, in_=prs)
            yn = sb.tile([128, 16, NPOS], BF16)
            for k in range(16):
                nc.vector.scalar_tensor_tensor(out=yn[:, k, :], in0=y_sb[:, k, :],
                                               scalar=nw_sb[:, k:k + 1], in1=r_bc,
                                               op0=OP.mult, op1=OP.mult)
            # ---- h = w_outT.T @ yn   [1024, 32]  (streamed weights)
            h_sb = sb.tile([128, 8, NPOS], BF16)
            for mt in range(8):
                if mt + 2 < 8:
                    load_wo(mt + 2)
                ph = psA.tile([128, NPOS], F32, tag="psh", bufs=3)
                for k in range(16):
                    nc.tensor.matmul(ph, wo_tiles[mt][:, k, :],
                                     yn[:, k, :], start=(k == 0), stop=(k == 15))
                nc.vector.tensor_copy(out=h_sb[:, mt, :], in_=ph)
            # ---- g = gelu(w1T.T @ h + b1)  [4096, 32]
            g_sb = sb.tile([128, 32, NPOS], BF16)
            for mt in range(32):
                if mt + 2 < 32:
                    load_w1(mt + 2)
                elif mt == 30:
                    load_w2(0)
                pg = psA.tile([128, NPOS], F32, tag="psh", bufs=3)
                for k in range(KT):
                    nc.tensor.matmul(pg, w1_tiles[mt][:, k, :],
                                     h_sb[:, k, :], start=(k == 0),
                                     stop=(k == KT - 1))
                nc.scalar.activation(out=g_sb[:, mt, :], in_=pg, func=AF.Gelu,
                                     bias=b1_sb[:, mt:mt + 1], scale=1.0)
            # ---- out = w2T.T @ g + b2   [512, 32] per core
            for mt in range(4):
                if mt + 1 < 4:
                    load_w2(mt + 1)
                po = psA.tile([128, NPOS], F32, tag="psh", bufs=3)
                for k in range(32):
                    nc.tensor.matmul(po, w2_tiles[mt][:, k, :],
                                     g_sb[:, k, :], start=(k == 0), stop=(k == 31))
                ot = sb.tile([128, NPOS], F32, tag="ot", bufs=2)
                nc.vector.tensor_scalar(ot, po, b2_sb[:, mt:mt + 1], None, OP.add)
                nc.sync.dma_start(out=out32[:, mt, :], in_=ot)

    nc.compile()
    _NC = nc
    return nc


# ----------------------------------------------------------------------------
# Host-side prep + glue
# ----------------------------------------------------------------------------

def _prep_maps(inputs):
    x = _f32(inputs["x"]).reshape(L, D_MODEL)
    xT = np.ascontiguousarray(x.T)                       # [1024, 8192]
    xT_b = _bf(xT)
    pos = np.arange(NPOS) * POS_STRIDE
    xTpos = _bf(xT[:, pos])
    # window tokens (pos, d): t*-3+d, zero-padded below 0
    win_idx = (pos[:, None] + np.arange(D_CONV)[None, :] - (D_CONV - 1)).reshape(-1)
    xTwin = np.zeros((D_MODEL, NPOS * D_CONV), np.float32)
    valid = win_idx >= 0
    xTwin[:, valid] = xT[:, win_idx[valid]]
    xTwin = _bf(xTwin)

    w_all = _f32(inputs["in_proj_w"])                    # [4384, 1024]
    conv_w = _f32(inputs["conv_w"])                      # [2304, 4]
    conv_b = _f32(inputs["conv_b"])                      # [2304]
    dt_bias = _f32(inputs["dt_bias"])                    # [32]
    A = -np.exp(_f32(inputs["A_log"]))                   # [32]
    Dp = _f32(inputs["D"])                               # [32]

    w_cT = _bf(w_all[D_INNER + D_INNER + D_STATE:
                     D_INNER + D_INNER + 2 * D_STATE].T.reshape(KT, 128, 128))
    cw_c = _f32(conv_w[D_INNER + D_STATE:])              # [128, 4] C channels
    conv_b_c = _f32(conv_b[D_INNER + D_STATE:]).reshape(128, 1)

    # one-hot head->partition broadcast map: oh[h, t*128+p] = (h == 2t + p//64)
    oh = np.zeros((HPC, 256), np.float32)
    for t in range(2):
        for p in range(128):
            oh[2 * t + p // 64, t * 128 + p] = 1.0
    oh = _bf(oh)

    # MLP tail (shared across cores except w2/b2)
    nw = _f32(inputs["norm_w"]).reshape(16, 128).transpose(1, 0).copy()
    w_out = _f32(inputs["mamba_out_w"])                  # [1024, 2048]
    w_outT = _bf(w_out.T.reshape(16, 128, D_MODEL))
    w1 = _f32(inputs["mlp_w1"])                          # [4096, 1024]
    w1T = _bf(w1.T.reshape(KT, 128, HIDDEN))
    b1 = _f32(inputs["mlp_b1"]).reshape(32, 128).transpose(1, 0).copy()
    w2 = _f32(inputs["mlp_w2"])                          # [4096, 4096]

    maps = []
    for k in range(NCORES):
        xs = 256 * k
        cols = np.concatenate([
            np.arange(D_INNER + xs, D_INNER + xs + 256),          # x slice
            np.arange(2 * D_INNER, 2 * D_INNER + D_STATE),        # B
        ])
        w_in = _bf(w_all[cols].T.reshape(KT, 128, NCOL))
        dt_cols = np.arange(D_IN_PROJ - NHEADS + HPC * k,
                            D_IN_PROJ - NHEADS + HPC * k + HPC)
        w_dt = _bf(w_all[dt_cols].T.reshape(KT, 128, HPC))
        w_z = _bf(w_all[xs:xs + 256].T.reshape(KT, 128, 256))
        # conv channels for this core: x slice (256) + B (128)
        ch_x = np.arange(xs, xs + 256)
        ch_B = np.arange(D_INNER, D_INNER + D_STATE)
        dw = np.zeros((3, D_CONV, 128, 128), np.float32)
        cb = np.zeros((128, 3), np.float32)
        for cht, chs in enumerate([ch_x[:128], ch_x[128:], ch_B]):
            for j in range(D_CONV):
                dw[cht, j] = np.diag(conv_w[chs, j])
            cb[:, cht] = conv_b[chs]
        heads = np.arange(HPC * k, HPC * k + HPC)
        colsl = slice(512 * k, 512 * k + 512)
        w2T = _bf(w2[colsl].T.reshape(32, 128, 512))
        b2 = _f32(inputs["mlp_b2"])[colsl].reshape(4, 128).transpose(1, 0).copy()
        maps.append({
            "xT": xT_b, "xTpos": xTpos, "xTwin": xTwin,
            "w_in": w_in, "w_dt": w_dt, "w_c": w_cT, "w_z": w_z,
            "diag_w": _bf(dw), "cw_c": cw_c, "conv_b": cb,
            "conv_b_c": conv_b_c,
            "dtb4": dt_bias[heads].reshape(HPC, 1).astype(np.float32),
            "A4": A[heads].reshape(HPC, 1).astype(np.float32),
            "D4": Dp[heads].reshape(HPC, 1).astype(np.float32),
            "oh_w": oh,
            "norm_w": nw, "w_outT": w_outT, "w1T": w1T, "b1": b1,
            "w2T": w2T, "b2": b2,
        })
    return maps


LAST_RESULTS = []


def kernel(**inputs) -> np.ndarray:
    trace = os.environ.get("KERNEL_TRACE", "0") == "1"
    LAST_RESULTS.clear()
    nc = build()
    maps = _prep_maps(inputs)
    res = bass_utils.run_bass_kernel_spmd(nc, maps, core_ids=list(range(NCORES)),
                                          trace=trace)
    LAST_RESULTS.append(res)
    out = np.zeros((NPOS, HIDDEN), np.float32)
    for k in range(NCORES):
        o = res.results[k]["out32"]                     # [128, 4, 32]
        # out[pos, 512k + mt*128 + p] = o[p, mt, pos]
        out[:, 512 * k:512 * (k + 1)] = o.transpose(2, 1, 0).reshape(NPOS, 512)
    return out.astype(np.float32)


# revision 16
# speedup vs baseline: 1.1894x; 1.1729x over previous
"""Trainium2 Bass kernel for nn_Connection_75411035783724 (Mamba2 block + MLP head).

Single fused launch, tensor-parallel over the 32 Mamba2 heads across 8 cores
(4 heads each).  Per core: in_proj column-slice (x-channels + B + dt), causal
depthwise conv as accumulating diagonal matmuls, chunked-SSD scan (chunk 256).
The gated RMSNorm + out_proj + MLP tail runs in the same NEFF: each core
computes its out_proj partial on un-normalized gated outputs (the rsqrt
factors out of the contraction), one AllReduce sums partials + sum-of-squares,
then the normalizer is applied and the MLP (MLP2 column-sharded) finishes.

Schedule notes:
- conv matmuls for group g-1 are emitted after in_proj for group g so the PE
  never waits on same-group PSUM evictions; scan matmuls lag one batch.
- the decay pipe runs per 1024-token batch entirely in the natural_log_exp
  activation-table set (softplus = ln(1+exp)); conv-silu is the only other
  scalar table in the loop.
- partition-broadcasts ([4]->[128] etc.) are one-hot matmuls.
- every DRAM operand is host-pre-tiled to its exact SBUF layout so DMA loads
  are contiguous per partition (descriptor-count, not bandwidth, dominated
  the naive layouts).
- MLP1/MLP2 weights stream through small SBUF rings, prefetched under the
  AllReduce.
"""
import os
import sys
import numpy as np
import ml_dtypes

sys.path.insert(0, "/opt/trn_rl_repo")

import concourse.bass as bass
import concourse.tile as tile
import concourse.mybir as mybir
from concourse import bacc
from concourse import bass_utils

F32 = mybir.dt.float32
BF16 = mybir.dt.bfloat16
AF = mybir.ActivationFunctionType
OP = mybir.AluOpType
BF = ml_dtypes.bfloat16

# Model dims
D_MODEL = 1024
HIDDEN = 4096
D_STATE = 128       # n
D_CONV = 4
D_INNER = 2048
HEADDIM = 64        # p
NHEADS = 32
CONV_DIM = D_INNER + 2 * D_STATE            # 2304
D_IN_PROJ = 2 * D_INNER + 2 * D_STATE + NHEADS  # 4384
L = 8192            # tokens
NPOS = 32           # output positions (first token of each frame)
POS_STRIDE = 256
NCORES = 8
HPC = 4             # heads per core
Q = 256             # chunk length
NCHUNK = L // Q     # 32
KT = D_MODEL // 128  # 8 K-tiles
NG = 16             # token groups of 512
GSZ = 512
BSZ = 2 * GSZ       # 1024-token batches for the decay pipe
NB = L // BSZ       # 8
CPB = BSZ // Q      # 4 chunks per batch
NCOL = 256 + 128  # 384: [x 256 | B 128]
MT_SPEC = [(0, 128), (128, 128), (256, 128)]  # (col0, width)
SFLAT = HPC * HEADDIM * NCHUNK  # 8192


def _bf(x):
    return np.ascontiguousarray(np.asarray(x, dtype=np.float32)).astype(BF)


def _f32(x):
    return np.ascontiguousarray(np.asarray(x, dtype=np.float32))


_NC = None


def build():
    global _NC
    if _NC is not None:
        return _NC
    nc = bacc.Bacc("TRN2", target_bir_lowering=False, debug=False,
                   num_devices=NCORES)

    def din(name, shape, dt):
        return nc.dram_tensor(name, shape, dt, kind="ExternalInput").ap()

    xT = din("xT", (NG, 128, KT, GSZ), BF16)
    xTpos = din("xTpos", (128, KT, NPOS), BF16)
    xTwin = din("xTwin", (128, KT, NPOS * D_CONV), BF16)
    w_in = din("w_in", (128, KT, NCOL), BF16)
    w_dt = din("w_dt", (128, KT, HPC), BF16)
    w_c = din("w_c", (128, KT, 128), BF16)
    w_z = din("w_z", (128, KT, 256), BF16)
    diag_w = din("diag_w", (128, 3, D_CONV, 128), BF16)
    cw_c = din("cw_c", (128, D_CONV), F32)
    conv_b = din("conv_b", (128, 3), F32)
    conv_b_c = din("conv_b_c", (128, 1), F32)
    dtb4 = din("dtb4", (HPC, 1), F32)
    A4 = din("A4", (HPC, 1), F32)
    D4 = din("D4", (HPC, 1), F32)
    oh_w = din("oh_w", (HPC, 256), BF16)        # one-hot head->partition map
    # tail
    nwl = din("nwl", (128, 2), F32)             # local norm weights
    wol = din("wol", (128, 2, D_MODEL), BF16)   # local out_proj rows
    w1 = din("w1T", (32, 128, KT, 128), BF16)
    b1 = din("b1", (128, HIDDEN // 128), F32)
    w2 = din("w2T", (4, 128, 32, 128), BF16)
    b2 = din("b2", (128, 4), F32)
    out32 = nc.dram_tensor("out32", (128, 4, NPOS), F32,
                           kind="ExternalOutput").ap()

    with tile.TileContext(nc) as tc:
        import contextlib
        with contextlib.ExitStack() as ctx:
            sb = ctx.enter_context(tc.tile_pool(name="sb", bufs=1))
            ring = ctx.enter_context(tc.tile_pool(name="ring", bufs=1))
            dsc = ctx.enter_context(tc.tile_pool(name="dsc", bufs=1, space="DRAM"))
            psA = ctx.enter_context(tc.tile_pool(name="psA", bufs=1, space="PSUM"))

            # ---- resident weights/constants (all pre-tiled, contiguous loads)
            w_in_sb = sb.tile([128, KT, NCOL], BF16)
            nc.sync.dma_start(out=w_in_sb, in_=w_in)
            w_dt_sb = sb.tile([128, KT, HPC], BF16)
            nc.sync.dma_start(out=w_dt_sb, in_=w_dt)
            w_c_sb = sb.tile([128, KT, 128], BF16)
            nc.sync.dma_start(out=w_c_sb, in_=w_c)
            w_z_sb = sb.tile([128, KT, 256], BF16)
            nc.sync.dma_start(out=w_z_sb, in_=w_z)
            diag_sb = sb.tile([128, 3, D_CONV, 128], BF16)
            nc.sync.dma_start(out=diag_sb, in_=diag_w)
            cw_sb = sb.tile([128, D_CONV], F32)
            nc.sync.dma_start(out=cw_sb, in_=cw_c)
            cb_sb = sb.tile([128, 3], F32)
            nc.sync.dma_start(out=cb_sb, in_=conv_b)
            cbc_sb = sb.tile([128, 1], F32)
            nc.sync.dma_start(out=cbc_sb, in_=conv_b_c)
            dtb_sb = sb.tile([HPC, 1], F32)
            nc.sync.dma_start(out=dtb_sb, in_=dtb4)
            A_sb = sb.tile([HPC, 1], F32)
            nc.sync.dma_start(out=A_sb, in_=A4)
            D_sb = sb.tile([HPC, 1], F32)
            nc.sync.dma_start(out=D_sb, in_=D4)
            oh_sb = sb.tile([HPC, 256], BF16)
            nc.sync.dma_start(out=oh_sb, in_=oh_w)
            xtp_sb = sb.tile([128, KT, NPOS], BF16)
            nc.sync.dma_start(out=xtp_sb, in_=xTpos)
            xtw_sb = sb.tile([128, KT, NPOS * D_CONV], BF16)
            nc.sync.dma_start(out=xtw_sb, in_=xTwin)
            nwl_sb = sb.tile([128, 2], F32)
            nc.sync.dma_start(out=nwl_sb, in_=nwl)
            wol_sb = sb.tile([128, 2, D_MODEL], BF16)
            nc.scalar.dma_start(out=wol_sb, in_=wol)
            b1_sb = sb.tile([128, HIDDEN // 128], F32)
            nc.sync.dma_start(out=b1_sb, in_=b1)
            b2_sb = sb.tile([128, 4], F32)
            nc.sync.dma_start(out=b2_sb, in_=b2)

            # ---- persistent big buffers
            xbcc = sb.tile([128, 3, L], BF16, tag="big1")  # conv+silu [x0|x1|B]
            XT = sb.tile([128, L // 128, 256], BF16, tag="big2")  # transposed w*x
            BT = sb.tile([128, L // 128, 128], BF16, tag="bt")    # transposed B
            S_all = sb.tile([128, SFLAT], BF16)      # per-chunk states (h,p,c)
            lamA = sb.tile([HPC, NCHUNK], F32)       # per-chunk decay
            dAA = sb.tile([HPC, NCHUNK], F32)        # exp(a) at chunk starts
            dt_pos = sb.tile([HPC, NCHUNK], F32)     # dt at chunk starts
            x32 = sb.tile([128, 2, NPOS], F32)
            B32 = sb.tile([128, NPOS], F32)
            ones4 = sb.tile([HPC, BSZ], F32)
            one4 = sb.tile([HPC, 1], F32)
            dt2_slots = [sb.tile([HPC, BSZ], F32, tag="dt2a", name="dt2a"),
                         sb.tile([HPC, BSZ], F32, tag="dt2b", name="dt2b")]
            a2_s = sb.tile([HPC, BSZ], F32)
            s2_s = sb.tile([HPC, BSZ], F32)
            w2b_s = sb.tile([HPC, BSZ], BF16)
            lam16 = sb.tile([HPC, NCHUNK], BF16)
            lam_d = dsc.tile([HPC, NCHUNK], BF16)
            nc.vector.memset(ones4, 1.0)
            nc.vector.memset(one4, 1.0)
            z_ap = bass.AP(tensor=ones4.tensor, offset=ones4.offset,
                           ap=[list(ones4.ap[0]), [Q, CPB]])
            nc.vector.memset(z_ap, 0.0)

            # ================= pre-loop tail-independent work ================
            C32 = sb.tile([128, NPOS], F32)
            pcw = psA.tile([128, NPOS * D_CONV], F32, tag="pin0")
            for k in range(KT):
                nc.tensor.matmul(pcw, w_c_sb[:, k, :], xtw_sb[:, k, :],
                                 start=(k == 0), stop=(k == KT - 1))
            tmpc = sb.tile([128, NPOS], F32)
            for j in range(D_CONV):
                src = bass.AP(tensor=pcw.tensor, offset=pcw.offset + j,
                              ap=[list(pcw.ap[0]), [D_CONV, NPOS]])
                if j == 0:
                    nc.vector.tensor_scalar_mul(tmpc, src, cw_sb[:, 0:1])
                else:
                    nc.vector.scalar_tensor_tensor(
                        out=tmpc, in0=src, scalar=cw_sb[:, j:j + 1], in1=tmpc,
                        op0=OP.mult, op1=OP.add)
            nc.scalar.activation(out=C32, in_=tmpc, func=AF.Silu,
                                 bias=cbc_sb[:, 0:1], scale=1.0)
            C32b = sb.tile([128, NPOS], BF16)
            nc.vector.tensor_copy(out=C32b, in_=C32)
            pz = psA.tile([128, 2, NPOS], F32, tag="pin1")
            for t in range(2):
                for k in range(KT):
                    nc.tensor.matmul(pz[:, t, :],
                                     w_z_sb[:, k, t * 128:(t + 1) * 128],
                                     xtp_sb[:, k, :],
                                     start=(k == 0), stop=(k == KT - 1))
            zs = sb.tile([128, 2, NPOS], F32)
            nc.scalar.activation(out=zs, in_=pz, func=AF.Silu)

            # ================= main fused loop =================
            xt_tiles = {}

            def load_xt(g):
                t = ring.tile([128, KT, GSZ], BF16, tag="xt", bufs=2)
                nc.sync.dma_start(out=t, in_=xT[g])
                xt_tiles[g] = t

            load_xt(0)
            load_xt(1)

            pins = {}
            pdts = {}
            xbc_tiles = {}

            def emit_inproj(g):
                xt_g = xt_tiles.pop(g)
                ps = []
                for mt, (c0, cw) in enumerate(MT_SPEC):
                    p = psA.tile([cw, GSZ], F32, tag=f"pin{mt}")
                    for k in range(KT):
                        nc.tensor.matmul(p, w_in_sb[:, k, c0:c0 + cw],
                                         xt_g[:, k, :],
                                         start=(k == 0), stop=(k == KT - 1))
                    ps.append(p)
                pdt = psA.tile([HPC, GSZ], F32, tag="pdt", bufs=2)
                for k in range(KT):
                    nc.tensor.matmul(pdt, w_dt_sb[:, k, :], xt_g[:, k, :],
                                     start=(k == 0), stop=(k == KT - 1))
                pins[g] = ps
                pdts[g] = pdt

            def emit_evict(g):
                ps = pins.pop(g)
                xbc_g = ring.tile([128, 3, GSZ + 3], BF16, tag="xbc", bufs=2)
                if g == 0:
                    nc.vector.memset(xbc_g[:, :, 0:3], 0.0)
                else:
                    prev = xbc_tiles[g - 1]
                    nc.vector.tensor_copy(out=xbc_g[:, :, 0:3],
                                          in_=prev[:, :, GSZ:GSZ + 3])
                for cht in range(3):
                    if cht != 2:
                        nc.vector.tensor_copy(out=xbc_g[:, cht, 3:], in_=ps[cht])
                    else:
                        nc.scalar.copy(out=xbc_g[:, cht, 3:], in_=ps[cht])
                xbc_tiles[g] = xbc_g
                b, half = divmod(g, 2)
                dt2 = dt2_slots[b % 2]
                pdt = pdts.pop(g)
                nc.vector.tensor_copy(out=dt2[:, half * GSZ:(half + 1) * GSZ],
                                      in_=pdt)

            def emit_conv(g):
                xbc_g = xbc_tiles[g]
                sl = slice(g * GSZ, (g + 1) * GSZ)
                for cht in range(3):
                    pc = psA.tile([128, GSZ], F32, tag="psh", bufs=3)
                    for j in range(D_CONV):
                        nc.tensor.matmul(pc, diag_sb[:, cht, j, :],
                                         xbc_g[:, cht, j:j + GSZ],
                                         start=(j == 0), stop=(j == D_CONV - 1))
                    nc.scalar.activation(out=xbcc[:, cht, sl], in_=pc,
                                         func=AF.Silu,
                                         bias=cb_sb[:, cht:cht + 1], scale=1.0)

            def emit_decay(b):
                # all scalar ops below live in the natural_log_exp table set
                dt2 = dt2_slots[b % 2]
                # softplus: dt = ln(1 + exp(v + bias))
                nc.scalar.activation(out=a2_s, in_=dt2, func=AF.Exp,
                                     bias=dtb_sb[:, 0:1], scale=1.0)
                nc.scalar.activation(out=dt2, in_=a2_s, func=AF.Ln,
                                     bias=one4[:, 0:1], scale=1.0)
                a2 = a2_s
                nc.vector.tensor_scalar_mul(a2, dt2, A_sb[:, 0:1])
                s2 = s2_s
                nc.vector.tensor_tensor_scan(out=s2, data0=ones4, data1=a2,
                                             initial=0.0, op0=OP.mult, op1=OP.add)
                cpos = b * CPB
                src = bass.AP(tensor=a2.tensor, offset=a2.offset,
                              ap=[list(a2.ap[0]), [Q, CPB]])
                nc.scalar.activation(out=dAA[:, cpos:cpos + CPB], in_=src,
                                     func=AF.Exp)
                src = bass.AP(tensor=dt2.tensor, offset=dt2.offset,
                              ap=[list(dt2.ap[0]), [Q, CPB]])
                nc.vector.tensor_copy(out=dt_pos[:, cpos:cpos + CPB], in_=src)
                src = bass.AP(tensor=s2.tensor, offset=s2.offset + Q - 1,
                              ap=[list(s2.ap[0]), [Q, CPB]])
                nc.scalar.activation(out=lamA[:, cpos:cpos + CPB], in_=src,
                                     func=AF.Exp)
                # incremental bf16 copy + DRAM stage of lambda for the tail
                if b == 0:
                    nc.vector.memset(lam16[:, 0:1], 0.0)
                    nc.vector.tensor_copy(out=lam16[:, 1:CPB],
                                          in_=lamA[:, 1:CPB])
                else:
                    nc.vector.tensor_copy(out=lam16[:, cpos:cpos + CPB],
                                          in_=lamA[:, cpos:cpos + CPB])
                nc.gpsimd.dma_start(out=lam_d[:, cpos:cpos + CPB],
                                    in_=lam16[:, cpos:cpos + CPB])
                # w = exp(stot - s) * dt
                for cc in range(CPB):
                    stot = bass.AP(tensor=s2.tensor,
                                   offset=s2.offset + cc * Q + Q - 1,
                                   ap=[list(s2.ap[0]), [1, 1]])
                    nc.vector.tensor_scalar(s2[:, cc * Q:(cc + 1) * Q],
                                            s2[:, cc * Q:(cc + 1) * Q],
                                            stot, None, OP.subtract)
                nc.scalar.activation(out=s2, in_=s2, func=AF.Exp, scale=-1.0)
                nc.vector.tensor_mul(w2b_s, s2, dt2)
                return w2b_s

            def emit_scale(b, w2b):
                bsl = slice(b * BSZ, (b + 1) * BSZ)
                for half in range(2):
                    hsl = slice(half * GSZ, (half + 1) * GSZ)
                    for t in range(2):
                        pw = psA.tile([128, GSZ], F32, tag="psh", bufs=3)
                        nc.tensor.matmul(pw, oh_sb[:, t * 128:(t + 1) * 128],
                                         w2b[:, hsl], start=True, stop=True)
                        xs = ring.tile([128, GSZ], BF16, tag="xs", bufs=2)
                        nc.vector.tensor_mul(
                            xs, xbcc[:, t, b * BSZ + half * GSZ:
                                     b * BSZ + (half + 1) * GSZ], pw)
                        eng = nc.sync if t == 0 else nc.scalar
                        eng.dma_start_transpose(
                            out=XT[:, 8 * b + 4 * half:8 * b + 4 * (half + 1),
                                   t * 128:(t + 1) * 128],
                            in_=xs)
                nc.sync.dma_start_transpose(out=BT[:, 8 * b:8 * (b + 1), :],
                                            in_=xbcc[:, 2, bsl])
                # position extracts for this batch (4 positions per batch)
                for cht in range(2):
                    s_ap = bass.AP(tensor=xbcc.tensor,
                                   offset=xbcc.offset + cht * L + b * BSZ,
                                   ap=[list(xbcc.ap[0]), [POS_STRIDE, CPB]])
                    nc.scalar.copy(out=x32[:, cht, CPB * b:CPB * (b + 1)],
                                   in_=s_ap)
                s_ap = bass.AP(tensor=xbcc.tensor,
                               offset=xbcc.offset + 2 * L + b * BSZ,
                               ap=[list(xbcc.ap[0]), [POS_STRIDE, CPB]])
                nc.scalar.copy(out=B32[:, CPB * b:CPB * (b + 1)], in_=s_ap)

            def emit_scan(b):
                for cc in range(CPB):
                    c = b * CPB + cc
                    pc2 = psA.tile([128, HPC * HEADDIM], F32, tag="psh", bufs=3)
                    for k2 in range(2):
                        T = 2 * c + k2
                        nc.tensor.matmul(pc2, BT[:, T, :], XT[:, T, :],
                                         start=(k2 == 0), stop=(k2 == 1))
                    dst = bass.AP(tensor=S_all.tensor, offset=S_all.offset + c,
                                  ap=[list(S_all.ap[0]), [NCHUNK, HPC * HEADDIM]])
                    if c % 2 == 0:
                        nc.vector.tensor_copy(out=dst, in_=pc2)
                    else:
                        nc.scalar.copy(out=dst, in_=pc2)

            for g in range(NG + 2):
                if g + 2 < NG:
                    load_xt(g + 2)
                if g < NG:
                    emit_inproj(g)
                    emit_evict(g)
                if g >= 1 and g - 1 < NG:
                    emit_conv(g - 1)
                if g >= 2 and g % 2 == 0:
                    b = (g - 2) // 2
                    emit_scale(b, emit_decay(b))
                if g >= 3 and g % 2 == 1:
                    emit_scan((g - 3) // 2)

            # ================= tail =================
            # lam_flat via DRAM->DRAM expand + partition-broadcast read
            lam_rd = dsc.tile([1, SFLAT], BF16)
            lam_src = bass.AP(tensor=lam_d.tensor, offset=lam_d.offset,
                              ap=[[0, 1], [NCHUNK, HPC], [0, HEADDIM],
                                  [1, NCHUNK]])
            nc.gpsimd.dma_start(
                out=lam_rd.rearrange("o (h p c) -> o h p c", h=HPC, p=HEADDIM),
                in_=lam_src)
            lam_flat = sb.tile([128, SFLAT], BF16, tag="bt")
            nc.gpsimd.dma_start(
                out=lam_flat,
                in_=bass.AP(tensor=lam_rd.tensor, offset=lam_rd.offset,
                            ap=[[0, 128], [1, SFLAT]]))

            # f4 = dt_pos * (B32 . C32) + D  (one-hot broadcasts via matmuls)
            bc_t = sb.tile([128, NPOS], BF16)
            nc.vector.tensor_mul(bc_t, B32, C32)
            onesc = sb.tile([128, 1], BF16)
            nc.vector.memset(onesc, 1.0)
            oh14 = sb.tile([1, HPC], BF16)
            nc.vector.memset(oh14, 1.0)
            ones128b = sb.tile([1, 128], BF16)
            nc.vector.memset(ones128b, 1.0)
            pbc = psA.tile([1, NPOS], F32, tag="pdt", bufs=2)
            nc.tensor.matmul(pbc, onesc, bc_t, start=True, stop=True)
            bc_row = sb.tile([1, NPOS], BF16)
            nc.vector.tensor_copy(out=bc_row, in_=pbc)
            pbc4 = psA.tile([HPC, NPOS], F32, tag="pdt", bufs=2)
            nc.tensor.matmul(pbc4, oh14, bc_row, start=True, stop=True)
            f4 = sb.tile([HPC, NPOS], F32)
            nc.vector.tensor_mul(f4, dt_pos, pbc4)
            nc.vector.tensor_scalar(f4, f4, D_sb[:, 0:1], None, OP.add)
            f4b = sb.tile([HPC, NPOS], BF16)
            nc.vector.tensor_copy(out=f4b, in_=f4)
            dAAb = sb.tile([HPC, NCHUNK], BF16)
            nc.vector.tensor_copy(out=dAAb, in_=dAA)
            pda = psA.tile([128, 2, NPOS], F32, tag="pin0")
            pf = psA.tile([128, 2, NPOS], F32, tag="pin1")
            for t in range(2):
                nc.tensor.matmul(pda[:, t, :], oh_sb[:, t * 128:(t + 1) * 128],
                                 dAAb, start=True, stop=True)
                nc.tensor.matmul(pf[:, t, :], oh_sb[:, t * 128:(t + 1) * 128],
                                 f4b, start=True, stop=True)
            dA_bc = sb.tile([128, 2, NPOS], F32)
            nc.vector.tensor_copy(out=dA_bc, in_=pda)
            f_bc = sb.tile([128, 2, NPOS], F32)
            nc.vector.tensor_copy(out=f_bc, in_=pf)

            # inter-chunk recurrence split by head pair; py matmuls interleave
            Sg = sb.tile([128, SFLAT], BF16, tag="big2")
            HALF = SFLAT // 2
            py = psA.tile([128, 2, NPOS], F32, tag="pin2")
            nc.vector.memset(py[:, :, 0:1], 0.0)
            for t in range(2):
                nc.vector.tensor_tensor_scan(
                    out=Sg[:, t * HALF:(t + 1) * HALF],
                    data0=lam_flat[:, t * HALF:(t + 1) * HALF],
                    data1=S_all[:, t * HALF:(t + 1) * HALF],
                    initial=0.0, op0=OP.mult, op1=OP.add)
                for pos in range(1, NPOS):
                    lhs = bass.AP(
                        tensor=Sg.tensor,
                        offset=Sg.offset + (2 * t) * HEADDIM * NCHUNK + (pos - 1),
                        ap=[list(Sg.ap[0]), [HEADDIM * NCHUNK, 2], [NCHUNK, 64]])
                    nc.tensor.matmul(py[:, t, pos:pos + 1], lhs,
                                     C32b[:, pos:pos + 1],
                                     start=(t == 0 and pos == 1),
                                     stop=(t == 1 and pos == NPOS - 1),
                                     skip_group_check=True)

            # y = (py*dA + x32*f)*silu(z), then y*norm_w (un-normalized)
            y32 = sb.tile([128, 2, NPOS], F32)
            nc.vector.tensor_mul(y32, py, dA_bc)
            tloc = sb.tile([128, 2, NPOS], F32)
            nc.vector.tensor_mul(tloc, x32, f_bc)
            nc.vector.tensor_add(y32, y32, tloc)
            nc.vector.tensor_mul(y32, y32, zs)
            sq2 = sb.tile([128, 2, NPOS], BF16)
            nc.vector.tensor_mul(sq2, y32, y32)
            ynwb = sb.tile([128, 2, NPOS], BF16)
            for t in range(2):
                nc.vector.tensor_scalar_mul(ynwb[:, t, :], y32[:, t, :],
                                            nwl_sb[:, t:t + 1])

            # local out_proj partials + sum-of-squares -> AllReduce buffer
            arb = dsc.tile([128, 9, NPOS], F32)
            for mt in range(8):
                php = psA.tile([128, NPOS], F32, tag="psh", bufs=3)
                for t in range(2):
                    nc.tensor.matmul(php, wol_sb[:, t, mt * 128:(mt + 1) * 128],
                                     ynwb[:, t, :], start=(t == 0), stop=(t == 1))
                hsb = sb.tile([128, NPOS], F32, tag="hp", bufs=3)
                if mt % 2 == 0:
                    nc.vector.tensor_copy(out=hsb, in_=php)
                else:
                    nc.scalar.copy(out=hsb, in_=php)
                nc.gpsimd.dma_start(out=arb[:, mt, :], in_=hsb)
            pss = psA.tile([1, NPOS], F32, tag="pdt", bufs=2)
            for t in range(2):
                nc.tensor.matmul(pss, onesc, sq2[:, t, :],
                                 start=(t == 0), stop=(t == 1))
            ssl = sb.tile([1, NPOS], F32)
            nc.vector.tensor_copy(out=ssl, in_=pss)
            nc.gpsimd.dma_start(out=arb[0:1, 8, :], in_=ssl)

            # prefetch first MLP1 weight tiles under the collective
            w1_tiles = [None] * 32

            def load_w1(mt):
                t = ring.tile([128, KT, 128], BF16, tag="w1r", bufs=4)
                nc.sync.dma_start(out=t, in_=w1[mt])
                w1_tiles[mt] = t

            w2_tiles = [None] * 4

            def load_w2(mt):
                t = ring.tile([128, 32, 128], BF16, tag="xt", bufs=2)
                nc.sync.dma_start(out=t, in_=w2[mt])
                w2_tiles[mt] = t

            for mt in range(4):
                load_w1(mt)

            arb_out = dsc.tile([128, 9, NPOS], F32)
            nc.gpsimd.collective_compute(
                "AllReduce", mybir.AluOpType.add,
                replica_groups=[list(range(NCORES))],
                ins=[arb.opt()], outs=[arb_out.opt()],
            )
            hsum = sb.tile([128, 9, NPOS], F32)
            nc.sync.dma_start(out=hsum, in_=arb_out)

            # r = 1/sqrt(mean + eps) = exp(-0.5*ln(mean + eps)); h = hsum*r
            eps_t = sb.tile([1, 1], F32)
            nc.vector.memset(eps_t, 1e-5)
            rs = sb.tile([1, NPOS], F32)
            nc.scalar.activation(out=rs, in_=hsum[0:1, 8, :], func=AF.Ln,
                                 bias=eps_t[:, 0:1], scale=1.0 / D_INNER)
            rsb = sb.tile([1, NPOS], BF16)
            nc.scalar.activation(out=rsb, in_=rs, func=AF.Exp, scale=-0.5)
            prs = psA.tile([128, NPOS], F32, tag="pdt", bufs=2)
            nc.tensor.matmul(prs, ones128b, rsb, start=True, stop=True)
            r_bc = sb.tile([128, NPOS], F32)
            nc.vector.tensor_copy(out=r_bc, in_=prs)
            h_sb = sb.tile([128, 8, NPOS], BF16)
            for k in range(KT):
                nc.vector.tensor_mul(h_sb[:, k, :], hsum[:, k, :], r_bc)

            # ---- g = gelu(w1T.T @ h + b1)  [4096, 32]
            g_sb = sb.tile([128, 32, NPOS], BF16)
            for mt in range(32):
                if mt + 4 < 32:
                    load_w1(mt + 4)
                elif mt == 28:
                    load_w2(0)
                elif mt == 29:
                    load_w2(1)
                pg = psA.tile([128, NPOS], F32, tag="psh", bufs=3)
                for k in range(KT):
                    nc.tensor.matmul(pg, w1_tiles[mt][:, k, :],
                                     h_sb[:, k, :], start=(k == 0),
                                     stop=(k == KT - 1))
                nc.scalar.activation(out=g_sb[:, mt, :], in_=pg, func=AF.Gelu,
                                     bias=b1_sb[:, mt:mt + 1], scale=1.0)
            # ---- out = w2T.T @ g + b2   [512, 32] per core
            for mt in range(4):
                if mt + 2 < 4:
                    load_w2(mt + 2)
                po = psA.tile([128, NPOS], F32, tag="psh", bufs=3)
                for k in range(32):
                    nc.tensor.matmul(po, w2_tiles[mt][:, k, :],
                                     g_sb[:, k, :], start=(k == 0), stop=(k == 31))
                ot = sb.tile([128, NPOS], F32, tag="ot", bufs=2)
                nc.vector.tensor_scalar(ot, po, b2_sb[:, mt:mt + 1], None, OP.add)
                nc.sync.dma_start(out=out32[:, mt, :], in_=ot)

    nc.compile()
    _NC = nc
    return nc


# ----------------------------------------------------------------------------
# Host-side prep + glue
# ----------------------------------------------------------------------------

def _prep_maps(inputs):
    x = _f32(inputs["x"]).reshape(L, D_MODEL)
    xT = np.ascontiguousarray(x.T)                       # [1024, 8192]
    # pre-tiled xT: [NG, 128, KT, GSZ]
    xT_t = _bf(xT.reshape(KT, 128, NG, GSZ).transpose(2, 1, 0, 3))
    pos = np.arange(NPOS) * POS_STRIDE
    xTpos = _bf(xT[:, pos].reshape(KT, 128, NPOS).transpose(1, 0, 2))
    win_idx = (pos[:, None] + np.arange(D_CONV)[None, :] - (D_CONV - 1)).reshape(-1)
    xTwin = np.zeros((D_MODEL, NPOS * D_CONV), np.float32)
    valid = win_idx >= 0
    xTwin[:, valid] = xT[:, win_idx[valid]]
    xTwin = _bf(xTwin.reshape(KT, 128, NPOS * D_CONV).transpose(1, 0, 2))

    w_all = _f32(inputs["in_proj_w"])                    # [4384, 1024]
    conv_w = _f32(inputs["conv_w"])                      # [2304, 4]
    conv_b = _f32(inputs["conv_b"])                      # [2304]
    dt_bias = _f32(inputs["dt_bias"])                    # [32]
    A = -np.exp(_f32(inputs["A_log"]))                   # [32]
    Dp = _f32(inputs["D"])                               # [32]

    w_cT = _bf(w_all[D_INNER + D_INNER + D_STATE:
                     D_INNER + D_INNER + 2 * D_STATE]
               .T.reshape(KT, 128, 128).transpose(1, 0, 2))
    cw_c = _f32(conv_w[D_INNER + D_STATE:])              # [128, 4] C channels
    conv_b_c = _f32(conv_b[D_INNER + D_STATE:]).reshape(128, 1)

    oh = np.zeros((HPC, 256), np.float32)
    for t in range(2):
        for p in range(128):
            oh[2 * t + p // 64, t * 128 + p] = 1.0
    oh = _bf(oh)

    norm_w = _f32(inputs["norm_w"])                      # [2048]
    w_out = _f32(inputs["mamba_out_w"])                  # [1024, 2048]
    w1 = _f32(inputs["mlp_w1"])                          # [4096, 1024]
    w1_t = _bf(w1.T.reshape(KT, 128, 32, 128).transpose(2, 1, 0, 3))
    b1 = _f32(inputs["mlp_b1"]).reshape(32, 128).transpose(1, 0).copy()
    w2 = _f32(inputs["mlp_w2"])                          # [4096, 4096]

    maps = []
    for k in range(NCORES):
        xs = 256 * k
        cols = np.concatenate([
            np.arange(D_INNER + xs, D_INNER + xs + 256),          # x slice
            np.arange(2 * D_INNER, 2 * D_INNER + D_STATE),        # B
        ])
        w_in = _bf(w_all[cols].T.reshape(KT, 128, NCOL).transpose(1, 0, 2))
        dt_cols = np.arange(D_IN_PROJ - NHEADS + HPC * k,
                            D_IN_PROJ - NHEADS + HPC * k + HPC)
        w_dtp = _bf(w_all[dt_cols].T.reshape(KT, 128, HPC).transpose(1, 0, 2))
        w_z = _bf(w_all[xs:xs + 256].T.reshape(KT, 128, 256).transpose(1, 0, 2))
        ch_x = np.arange(xs, xs + 256)
        ch_B = np.arange(D_INNER, D_INNER + D_STATE)
        dw = np.zeros((3, D_CONV, 128, 128), np.float32)
        cb = np.zeros((128, 3), np.float32)
        for cht, chs in enumerate([ch_x[:128], ch_x[128:], ch_B]):
            for j in range(D_CONV):
                dw[cht, j] = np.diag(conv_w[chs, j])
            cb[:, cht] = conv_b[chs]
        dw = _bf(dw.transpose(2, 0, 1, 3))               # [128, 3, 4, 128]
        heads = np.arange(HPC * k, HPC * k + HPC)
        # local channels of this core: ch = 256k + t*128 + p
        chl = (xs + np.arange(256)).reshape(2, 128)      # [t, p]
        nwl = norm_w[chl].T.copy()                       # [128, 2]
        wol = _bf(w_out[:, xs:xs + 256].T.reshape(2, 128, D_MODEL)
                  .transpose(1, 0, 2))                   # [128, 2, 1024]
        colsl = slice(512 * k, 512 * k + 512)
        w2_t = _bf(w2[colsl].T.reshape(32, 128, 4, 128).transpose(2, 1, 0, 3))
        b2 = _f32(inputs["mlp_b2"])[colsl].reshape(4, 128).transpose(1, 0).copy()
        maps.append({
            "xT": xT_t, "xTpos": xTpos, "xTwin": xTwin,
            "w_in": w_in, "w_dt": w_dtp, "w_c": w_cT, "w_z": w_z,
            "diag_w": dw, "cw_c": cw_c, "conv_b": cb,
            "conv_b_c": conv_b_c,
            "dtb4": dt_bias[heads].reshape(HPC, 1).astype(np.float32),
            "A4": A[heads].reshape(HPC, 1).astype(np.float32),
            "D4": Dp[heads].reshape(HPC, 1).astype(np.float32),
            "oh_w": oh,
            "nwl": nwl, "wol": wol, "w1T": w1_t, "b1": b1,
            "w2T": w2_t, "b2": b2,
        })
    return maps


LAST_RESULTS = []


def kernel(**inputs) -> np.ndarray:
    trace = os.environ.get("KERNEL_TRACE", "0") == "1"
    LAST_RESULTS.clear()
    nc = build()
    maps = _prep_maps(inputs)
    res = bass_utils.run_bass_kernel_spmd(nc, maps, core_ids=list(range(NCORES)),
                                          trace=trace)
    LAST_RESULTS.append(res)
    out = np.zeros((NPOS, HIDDEN), np.float32)
    for k in range(NCORES):
        o = res.results[k]["out32"]                     # [128, 4, 32]
        out[:, 512 * k:512 * (k + 1)] = o.transpose(2, 1, 0).reshape(NPOS, 512)
    return out.astype(np.float32)


# revision 19
# speedup vs baseline: 1.2169x; 1.0231x over previous
"""Trainium2 Bass kernel for nn_Connection_75411035783724 (Mamba2 block + MLP head).

Single fused launch, tensor-parallel over the 32 Mamba2 heads across 8 cores
(4 heads each).  Per core: in_proj column-slice (x-channels + B + dt), causal
depthwise conv as accumulating diagonal matmuls, chunked-SSD scan (chunk 256).
The gated RMSNorm + out_proj + MLP tail runs in the same NEFF: each core
computes its out_proj partial on un-normalized gated outputs (the rsqrt
factors out of the contraction), one AllReduce sums partials + sum-of-squares,
then the normalizer is applied and the MLP (MLP2 column-sharded) finishes.

Schedule notes:
- conv matmuls for group g-1 are emitted after in_proj for group g so the PE
  never waits on same-group PSUM evictions; scan matmuls lag one batch.
- the decay pipe runs per 1024-token batch entirely in the natural_log_exp
  activation-table set (softplus = ln(1+exp)); conv-silu is the only other
  scalar table in the loop.
- partition-broadcasts ([4]->[128] etc.) are one-hot matmuls.
- every DRAM operand is host-pre-tiled to its exact SBUF layout so DMA loads
  are contiguous per partition (descriptor-count, not bandwidth, dominated
  the naive layouts).
- MLP1/MLP2 weights stream through small SBUF rings, prefetched under the
  AllReduce.
"""
import os
import sys
import numpy as np
import ml_dtypes

sys.path.insert(0, "/opt/trn_rl_repo")

import concourse.bass as bass
import concourse.tile as tile
import concourse.mybir as mybir
from concourse import bacc
from concourse import bass_utils

F32 = mybir.dt.float32
BF16 = mybir.dt.bfloat16
AF = mybir.ActivationFunctionType
OP = mybir.AluOpType
BF = ml_dtypes.bfloat16

# Model dims
D_MODEL = 1024
HIDDEN = 4096
D_STATE = 128       # n
D_CONV = 4
D_INNER = 2048
HEADDIM = 64        # p
NHEADS = 32
CONV_DIM = D_INNER + 2 * D_STATE            # 2304
D_IN_PROJ = 2 * D_INNER + 2 * D_STATE + NHEADS  # 4384
L = 8192            # tokens
NPOS = 32           # output positions (first token of each frame)
POS_STRIDE = 256
NCORES = 8
HPC = 4             # heads per core
Q = 256             # chunk length
NCHUNK = L // Q     # 32
KT = D_MODEL // 128  # 8 K-tiles
NG = 16             # token groups of 512
GSZ = 512
BSZ = 2 * GSZ       # 1024-token batches for the decay pipe
NB = L // BSZ       # 8
CPB = BSZ // Q      # 4 chunks per batch
NCOL = 256 + 128  # 384: [x 256 | B 128]
MT_SPEC = [(0, 128), (128, 128), (256, 128)]  # (col0, width)
SFLAT = HPC * HEADDIM * NCHUNK  # 8192


def _bf(x):
    return np.ascontiguousarray(np.asarray(x, dtype=np.float32)).astype(BF)


def _f32(x):
    return np.ascontiguousarray(np.asarray(x, dtype=np.float32))


_NC = None


def build():
    global _NC
    if _NC is not None:
        return _NC
    nc = bacc.Bacc("TRN2", target_bir_lowering=False, debug=False,
                   num_devices=NCORES)

    def din(name, shape, dt):
        return nc.dram_tensor(name, shape, dt, kind="ExternalInput").ap()

    xT = din("xT", (NG, 128, KT, GSZ), BF16)
    xTpos = din("xTpos", (128, KT, NPOS), BF16)
    xTwin = din("xTwin", (128, KT, NPOS * D_CONV), BF16)
    w_in = din("w_in", (128, KT, NCOL), BF16)
    w_dt = din("w_dt", (128, KT, HPC), BF16)
    w_c = din("w_c", (128, KT, 128), BF16)
    w_z = din("w_z", (128, KT, 256), BF16)
    diag_w = din("diag_w", (128, 3, D_CONV, 128), BF16)
    cw_c = din("cw_c", (128, D_CONV), F32)
    conv_b = din("conv_b", (128, 3), F32)
    conv_b_c = din("conv_b_c", (128, 1), F32)
    dtb4 = din("dtb4", (HPC, 1), F32)
    A4 = din("A4", (HPC, 1), F32)
    D4 = din("D4", (HPC, 1), F32)
    oh_w = din("oh_w", (HPC, 256), BF16)        # one-hot head->partition map
    # tail
    nwl = din("nwl", (128, 2), F32)             # local norm weights
    wol = din("wol", (128, 2, D_MODEL), BF16)   # local out_proj rows
    w1 = din("w1T", (32, 128, KT, 128), BF16)
    b1 = din("b1", (128, HIDDEN // 128), F32)
    w2 = din("w2T", (4, 128, 32, 128), BF16)
    b2 = din("b2", (128, 4), F32)
    out32 = nc.dram_tensor("out32", (128, 4, NPOS), F32,
                           kind="ExternalOutput").ap()

    with tile.TileContext(nc) as tc:
        import contextlib
        with contextlib.ExitStack() as ctx:
            sb = ctx.enter_context(tc.tile_pool(name="sb", bufs=1))
            ring = ctx.enter_context(tc.tile_pool(name="ring", bufs=1))
            dsc = ctx.enter_context(tc.tile_pool(name="dsc", bufs=1, space="DRAM"))
            psA = ctx.enter_context(tc.tile_pool(name="psA", bufs=1, space="PSUM"))

            # ---- resident weights/constants (all pre-tiled, contiguous loads)
            w_in_sb = sb.tile([128, KT, NCOL], BF16)
            nc.sync.dma_start(out=w_in_sb, in_=w_in)
            w_dt_sb = sb.tile([128, KT, HPC], BF16)
            nc.sync.dma_start(out=w_dt_sb, in_=w_dt)
            xt_tiles = {}

            def load_xt(g):
                t = ring.tile([128, KT, GSZ], BF16, tag="xt", bufs=2)
                nc.sync.dma_start(out=t, in_=xT[g])
                xt_tiles[g] = t

            load_xt(0)
            load_xt(1)
            w_c_sb = sb.tile([128, KT, 128], BF16)
            nc.sync.dma_start(out=w_c_sb, in_=w_c)
            w_z_sb = sb.tile([128, KT, 256], BF16)
            nc.sync.dma_start(out=w_z_sb, in_=w_z)
            diag_sb = sb.tile([128, 3, D_CONV, 128], BF16)
            nc.sync.dma_start(out=diag_sb, in_=diag_w)
            cw_sb = sb.tile([128, D_CONV], F32)
            nc.sync.dma_start(out=cw_sb, in_=cw_c)
            cb_sb = sb.tile([128, 3], F32)
            nc.sync.dma_start(out=cb_sb, in_=conv_b)
            cbc_sb = sb.tile([128, 1], F32)
            nc.sync.dma_start(out=cbc_sb, in_=conv_b_c)
            dtb_sb = sb.tile([HPC, 1], F32)
            nc.sync.dma_start(out=dtb_sb, in_=dtb4)
            A_sb = sb.tile([HPC, 1], F32)
            nc.sync.dma_start(out=A_sb, in_=A4)
            D_sb = sb.tile([HPC, 1], F32)
            nc.sync.dma_start(out=D_sb, in_=D4)
            oh_sb = sb.tile([HPC, 256], BF16)
            nc.sync.dma_start(out=oh_sb, in_=oh_w)
            xtp_sb = sb.tile([128, KT, NPOS], BF16)
            nc.sync.dma_start(out=xtp_sb, in_=xTpos)
            xtw_sb = sb.tile([128, KT, NPOS * D_CONV], BF16)
            nc.sync.dma_start(out=xtw_sb, in_=xTwin)
            nwl_sb = sb.tile([128, 2], F32)
            nc.sync.dma_start(out=nwl_sb, in_=nwl)
            wol_sb = sb.tile([128, 2, D_MODEL], BF16)
            nc.scalar.dma_start(out=wol_sb, in_=wol)
            b1_sb = sb.tile([128, HIDDEN // 128], F32)
            nc.sync.dma_start(out=b1_sb, in_=b1)
            b2_sb = sb.tile([128, 4], F32)
            nc.sync.dma_start(out=b2_sb, in_=b2)

            # ---- persistent big buffers
            xbcc = sb.tile([128, 3, L], BF16, tag="big1")  # conv+silu [x0|x1|B]
            XT = sb.tile([128, L // 128, 256], BF16, tag="big2")  # transposed w*x
            BT = sb.tile([128, L // 128, 128], BF16, tag="bt")    # transposed B
            S_all = sb.tile([128, SFLAT], BF16)      # per-chunk states (h,p,c)
            lamA = sb.tile([HPC, NCHUNK], F32)       # per-chunk decay
            dAA = sb.tile([HPC, NCHUNK], F32)        # exp(a) at chunk starts
            dt_pos = sb.tile([HPC, NCHUNK], F32)     # dt at chunk starts
            x32 = sb.tile([128, 2, NPOS], F32)
            B32 = sb.tile([128, NPOS], F32)
            ones4 = sb.tile([HPC, BSZ], F32)
            one4 = sb.tile([HPC, 1], F32)
            dt2_slots = [sb.tile([HPC, BSZ], F32, tag="dt2a", name="dt2a"),
                         sb.tile([HPC, BSZ], F32, tag="dt2b", name="dt2b")]
            a2_s = sb.tile([HPC, BSZ], F32)
            s2_s = sb.tile([HPC, BSZ], F32)
            w2b_s = sb.tile([HPC, BSZ], BF16)
            lam16 = sb.tile([HPC, NCHUNK], BF16)
            lam_d = dsc.tile([HPC, NCHUNK], BF16)
            onesc = sb.tile([128, 1], BF16)
            oh14 = sb.tile([1, HPC], BF16)
            ones128b = sb.tile([1, 128], BF16)
            hp_all = sb.tile([128, 9, NPOS], BF16)
            nc.vector.memset(onesc, 1.0)
            nc.vector.memset(oh14, 1.0)
            nc.vector.memset(ones128b, 1.0)
            nc.vector.memset(hp_all[:, 8, :], 0.0)
            nc.vector.memset(ones4, 1.0)
            nc.vector.memset(one4, 1.0)
            z_ap = bass.AP(tensor=ones4.tensor, offset=ones4.offset,
                           ap=[list(ones4.ap[0]), [Q, CPB]])
            nc.vector.memset(z_ap, 0.0)

            # ================= pre-loop tail-independent work ================
            C32 = sb.tile([128, NPOS], F32)
            pcw = psA.tile([128, NPOS * D_CONV], F32, tag="pin0")
            for k in range(KT):
                nc.tensor.matmul(pcw, w_c_sb[:, k, :], xtw_sb[:, k, :],
                                 start=(k == 0), stop=(k == KT - 1))
            tmpc = sb.tile([128, NPOS], F32)
            for j in range(D_CONV):
                src = bass.AP(tensor=pcw.tensor, offset=pcw.offset + j,
                              ap=[list(pcw.ap[0]), [D_CONV, NPOS]])
                if j == 0:
                    nc.vector.tensor_scalar_mul(tmpc, src, cw_sb[:, 0:1])
                else:
                    nc.vector.scalar_tensor_tensor(
                        out=tmpc, in0=src, scalar=cw_sb[:, j:j + 1], in1=tmpc,
                        op0=OP.mult, op1=OP.add)
            nc.scalar.activation(out=C32, in_=tmpc, func=AF.Silu,
                                 bias=cbc_sb[:, 0:1], scale=1.0)
            C32b = sb.tile([128, NPOS], BF16)
            nc.vector.tensor_copy(out=C32b, in_=C32)
            pz = psA.tile([128, 2, NPOS], F32, tag="pin1")
            for t in range(2):
                for k in range(KT):
                    nc.tensor.matmul(pz[:, t, :],
                                     w_z_sb[:, k, t * 128:(t + 1) * 128],
                                     xtp_sb[:, k, :],
                                     start=(k == 0), stop=(k == KT - 1))
            zs = sb.tile([128, 2, NPOS], F32)
            nc.scalar.activation(out=zs, in_=pz, func=AF.Silu)

            # ================= main fused loop =================
            pins = {}
            pdts = {}
            xbc_tiles = {}

            def emit_inproj(g):
                xt_g = xt_tiles.pop(g)
                ps = []
                for mt, (c0, cw) in enumerate(MT_SPEC):
                    p = psA.tile([cw, GSZ], F32, tag=f"pin{mt}")
                    for k in range(KT):
                        nc.tensor.matmul(p, w_in_sb[:, k, c0:c0 + cw],
                                         xt_g[:, k, :],
                                         start=(k == 0), stop=(k == KT - 1))
                    ps.append(p)
                pdt = psA.tile([HPC, GSZ], F32, tag="pdt", bufs=2)
                for k in range(KT):
                    nc.tensor.matmul(pdt, w_dt_sb[:, k, :], xt_g[:, k, :],
                                     start=(k == 0), stop=(k == KT - 1))
                pins[g] = ps
                pdts[g] = pdt

            def emit_evict(g):
                ps = pins.pop(g)
                xbc_g = ring.tile([128, 3, GSZ + 3], BF16, tag="xbc", bufs=2)
                if g == 0:
                    nc.vector.memset(xbc_g[:, :, 0:3], 0.0)
                else:
                    prev = xbc_tiles[g - 1]
                    nc.vector.tensor_copy(out=xbc_g[:, :, 0:3],
                                          in_=prev[:, :, GSZ:GSZ + 3])
                for cht in range(3):
                    if cht != 2:
                        nc.vector.tensor_copy(out=xbc_g[:, cht, 3:], in_=ps[cht])
                    else:
                        nc.scalar.copy(out=xbc_g[:, cht, 3:], in_=ps[cht])
                xbc_tiles[g] = xbc_g
                b, half = divmod(g, 2)
                dt2 = dt2_slots[b % 2]
                pdt = pdts.pop(g)
                nc.vector.tensor_copy(out=dt2[:, half * GSZ:(half + 1) * GSZ],
                                      in_=pdt)

            def emit_conv(g):
                xbc_g = xbc_tiles[g]
                sl = slice(g * GSZ, (g + 1) * GSZ)
                for cht in range(3):
                    pc = psA.tile([128, GSZ], F32, tag="psh", bufs=3)
                    for j in range(D_CONV):
                        nc.tensor.matmul(pc, diag_sb[:, cht, j, :],
                                         xbc_g[:, cht, j:j + GSZ],
                                         start=(j == 0), stop=(j == D_CONV - 1))
                    nc.scalar.activation(out=xbcc[:, cht, sl], in_=pc,
                                         func=AF.Silu,
                                         bias=cb_sb[:, cht:cht + 1], scale=1.0)

            def emit_decay(b):
                # all scalar ops below live in the natural_log_exp table set
                dt2 = dt2_slots[b % 2]
                # softplus: dt = ln(1 + exp(v + bias))
                nc.scalar.activation(out=a2_s, in_=dt2, func=AF.Exp,
                                     bias=dtb_sb[:, 0:1], scale=1.0)
                nc.scalar.activation(out=dt2, in_=a2_s, func=AF.Ln,
                                     bias=one4[:, 0:1], scale=1.0)
                a2 = a2_s
                nc.vector.tensor_scalar_mul(a2, dt2, A_sb[:, 0:1])
                s2 = s2_s
                nc.vector.tensor_tensor_scan(out=s2, data0=ones4, data1=a2,
                                             initial=0.0, op0=OP.mult, op1=OP.add)
                cpos = b * CPB
                src = bass.AP(tensor=a2.tensor, offset=a2.offset,
                              ap=[list(a2.ap[0]), [Q, CPB]])
                nc.scalar.activation(out=dAA[:, cpos:cpos + CPB], in_=src,
                                     func=AF.Exp)
                src = bass.AP(tensor=dt2.tensor, offset=dt2.offset,
                              ap=[list(dt2.ap[0]), [Q, CPB]])
                nc.vector.tensor_copy(out=dt_pos[:, cpos:cpos + CPB], in_=src)
                src = bass.AP(tensor=s2.tensor, offset=s2.offset + Q - 1,
                              ap=[list(s2.ap[0]), [Q, CPB]])
                nc.scalar.activation(out=lamA[:, cpos:cpos + CPB], in_=src,
                                     func=AF.Exp)
                # incremental bf16 copy + DRAM stage of lambda for the tail
                if b == 0:
                    nc.vector.memset(lam16[:, 0:1], 0.0)
                    nc.vector.tensor_copy(out=lam16[:, 1:CPB],
                                          in_=lamA[:, 1:CPB])
                else:
                    nc.vector.tensor_copy(out=lam16[:, cpos:cpos + CPB],
                                          in_=lamA[:, cpos:cpos + CPB])
                nc.gpsimd.dma_start(out=lam_d[:, cpos:cpos + CPB],
                                    in_=lam16[:, cpos:cpos + CPB])
                # w = exp(stot - s) * dt
                for cc in range(CPB):
                    stot = bass.AP(tensor=s2.tensor,
                                   offset=s2.offset + cc * Q + Q - 1,
                                   ap=[list(s2.ap[0]), [1, 1]])
                    nc.vector.tensor_scalar(s2[:, cc * Q:(cc + 1) * Q],
                                            s2[:, cc * Q:(cc + 1) * Q],
                                            stot, None, OP.subtract)
                nc.scalar.activation(out=s2, in_=s2, func=AF.Exp, scale=-1.0)
                nc.vector.tensor_mul(w2b_s, s2, dt2)
                return w2b_s

            def emit_scale(b, w2b):
                bsl = slice(b * BSZ, (b + 1) * BSZ)
                for half in range(2):
                    hsl = slice(half * GSZ, (half + 1) * GSZ)
                    for t in range(2):
                        pw = psA.tile([128, GSZ], F32, tag="psh", bufs=3)
                        nc.tensor.matmul(pw, oh_sb[:, t * 128:(t + 1) * 128],
                                         w2b[:, hsl], start=True, stop=True)
                        xs = ring.tile([128, GSZ], BF16, tag="xs", bufs=2)
                        nc.vector.tensor_mul(
                            xs, xbcc[:, t, b * BSZ + half * GSZ:
                                     b * BSZ + (half + 1) * GSZ], pw)
                        eng = nc.sync if t == 0 else nc.scalar
                        eng.dma_start_transpose(
                            out=XT[:, 8 * b + 4 * half:8 * b + 4 * (half + 1),
                                   t * 128:(t + 1) * 128],
                            in_=xs)
                nc.sync.dma_start_transpose(out=BT[:, 8 * b:8 * (b + 1), :],
                                            in_=xbcc[:, 2, bsl])
                # position extracts for this batch (4 positions per batch)
                for cht in range(2):
                    s_ap = bass.AP(tensor=xbcc.tensor,
                                   offset=xbcc.offset + cht * L + b * BSZ,
                                   ap=[list(xbcc.ap[0]), [POS_STRIDE, CPB]])
                    nc.scalar.copy(out=x32[:, cht, CPB * b:CPB * (b + 1)],
                                   in_=s_ap)
                s_ap = bass.AP(tensor=xbcc.tensor,
                               offset=xbcc.offset + 2 * L + b * BSZ,
                               ap=[list(xbcc.ap[0]), [POS_STRIDE, CPB]])
                nc.scalar.copy(out=B32[:, CPB * b:CPB * (b + 1)], in_=s_ap)

            def emit_scan(b):
                for cc in range(CPB):
                    c = b * CPB + cc
                    pc2 = psA.tile([128, HPC * HEADDIM], F32, tag="psh", bufs=3)
                    for k2 in range(2):
                        T = 2 * c + k2
                        nc.tensor.matmul(pc2, BT[:, T, :], XT[:, T, :],
                                         start=(k2 == 0), stop=(k2 == 1))
                    dst = bass.AP(tensor=S_all.tensor, offset=S_all.offset + c,
                                  ap=[list(S_all.ap[0]), [NCHUNK, HPC * HEADDIM]])
                    if c % 2 == 0:
                        nc.vector.tensor_copy(out=dst, in_=pc2)
                    else:
                        nc.scalar.copy(out=dst, in_=pc2)

            dAzs = sb.tile([128, 2, NPOS], F32)
            tloc2 = sb.tile([128, 2, NPOS], F32)

            def emit_f4():
                # f4 = dt_pos*(B32.C32)+D; then pre-fold the z-gate:
                # y32 = py*(dA*zs) + (x32*f*zs)
                bc_t = sb.tile([128, NPOS], BF16)
                nc.vector.tensor_mul(bc_t, B32, C32)
                pbc = psA.tile([1, NPOS], F32, tag="pdt", bufs=2)
                nc.tensor.matmul(pbc, onesc, bc_t, start=True, stop=True)
                bc_row = sb.tile([1, NPOS], BF16)
                nc.scalar.copy(out=bc_row, in_=pbc)
                pbc4 = psA.tile([HPC, NPOS], F32, tag="pdt", bufs=2)
                nc.tensor.matmul(pbc4, oh14, bc_row, start=True, stop=True)
                f4 = sb.tile([HPC, NPOS], F32)
                nc.vector.tensor_mul(f4, dt_pos, pbc4)
                nc.vector.tensor_scalar(f4, f4, D_sb[:, 0:1], None, OP.add)
                f4b = sb.tile([HPC, NPOS], BF16)
                nc.scalar.copy(out=f4b, in_=f4)
                dAAb = sb.tile([HPC, NCHUNK], BF16)
                nc.scalar.copy(out=dAAb, in_=dAA)
                pda = psA.tile([128, 2, NPOS], F32, tag="pin0")
                pf = psA.tile([128, 2, NPOS], F32, tag="pin1")
                for t in range(2):
                    nc.tensor.matmul(pda[:, t, :],
                                     oh_sb[:, t * 128:(t + 1) * 128],
                                     dAAb, start=True, stop=True)
                    nc.tensor.matmul(pf[:, t, :],
                                     oh_sb[:, t * 128:(t + 1) * 128],
                                     f4b, start=True, stop=True)
                nc.vector.tensor_mul(dAzs, pda, zs)
                nc.vector.tensor_mul(tloc2, x32, pf)
                nc.vector.tensor_mul(tloc2, tloc2, zs)

            for g in range(NG + 2):
                if g + 2 < NG:
                    load_xt(g + 2)
                if g < NG:
                    emit_inproj(g)
                    emit_evict(g)
                if g >= 1 and g - 1 < NG:
                    emit_conv(g - 1)
                if g >= 2 and g % 2 == 0:
                    b = (g - 2) // 2
                    emit_scale(b, emit_decay(b))
                if g == NG:
                    emit_f4()
                if g >= 3 and g % 2 == 1:
                    emit_scan((g - 3) // 2)

            # ================= tail =================
            # preload first half of MLP weights into freed loop buffers
            w1pre = sb.tile([128, 16, KT, 128], BF16, tag="big1")
            nc.sync.dma_start(out=w1pre,
                              in_=w1[0:16].rearrange("m p k c -> p m k c"))
            w2pre = sb.tile([128, 2, 32, 128], BF16, tag="bt")
            nc.scalar.dma_start(out=w2pre,
                                in_=w2[0:2].rearrange("m p k c -> p m k c"))
            # lam_flat via DRAM->DRAM expand + partition-broadcast read
            lam_rd = dsc.tile([1, SFLAT], BF16)
            lam_src = bass.AP(tensor=lam_d.tensor, offset=lam_d.offset,
                              ap=[[0, 1], [NCHUNK, HPC], [0, HEADDIM],
                                  [1, NCHUNK]])
            nc.gpsimd.dma_start(
                out=lam_rd.rearrange("o (h p c) -> o h p c", h=HPC, p=HEADDIM),
                in_=lam_src)
            HALF = SFLAT // 2
            lam_halves = []
            for t in range(2):
                lf = ring.tile([128, HALF], BF16, tag="xt", bufs=2)
                nc.gpsimd.dma_start(
                    out=lf,
                    in_=bass.AP(tensor=lam_rd.tensor,
                                offset=lam_rd.offset + t * HALF,
                                ap=[[0, 128], [1, HALF]]))
                lam_halves.append(lf)

            # inter-chunk recurrence split by head pair; py matmuls interleave
            Sg = sb.tile([128, SFLAT], BF16, tag="big2")
            py = psA.tile([128, 2, NPOS], F32, tag="pin2")
            nc.vector.memset(py[:, :, 0:1], 0.0)
            for t in range(2):
                nc.vector.tensor_tensor_scan(
                    out=Sg[:, t * HALF:(t + 1) * HALF],
                    data0=lam_halves[t],
                    data1=S_all[:, t * HALF:(t + 1) * HALF],
                    initial=0.0, op0=OP.mult, op1=OP.add)
                for pos in range(1, NPOS):
                    lhs = bass.AP(
                        tensor=Sg.tensor,
                        offset=Sg.offset + (2 * t) * HEADDIM * NCHUNK + (pos - 1),
                        ap=[list(Sg.ap[0]), [HEADDIM * NCHUNK, 2], [NCHUNK, 64]])
                    nc.tensor.matmul(py[:, t, pos:pos + 1], lhs,
                                     C32b[:, pos:pos + 1],
                                     start=(t == 0 and pos == 1),
                                     stop=(t == 1 and pos == NPOS - 1),
                                     skip_group_check=True)

            # y = py*(dA*zs) + (x32*f*zs), then y*norm_w (un-normalized)
            y32 = sb.tile([128, 2, NPOS], F32)
            nc.vector.tensor_mul(y32, py, dAzs)
            nc.vector.tensor_add(y32, y32, tloc2)
            sq2 = sb.tile([128, 2, NPOS], BF16)
            nc.vector.tensor_mul(sq2, y32, y32)
            ynwb = sb.tile([128, 2, NPOS], BF16)
            for t in range(2):
                nc.vector.tensor_scalar_mul(ynwb[:, t, :], y32[:, t, :],
                                            nwl_sb[:, t:t + 1])

            # local out_proj partials + sum-of-squares -> AllReduce buffer
            arb = dsc.tile([128, 9, NPOS], BF16)
            for mt in range(8):
                php = psA.tile([128, NPOS], F32, tag="psh", bufs=3)
                for t in range(2):
                    nc.tensor.matmul(php, wol_sb[:, t, mt * 128:(mt + 1) * 128],
                                     ynwb[:, t, :], start=(t == 0), stop=(t == 1))
                if mt % 2 == 0:
                    nc.vector.tensor_copy(out=hp_all[:, mt, :], in_=php)
                else:
                    nc.scalar.copy(out=hp_all[:, mt, :], in_=php)
            pss = psA.tile([1, NPOS], F32, tag="pdt", bufs=2)
            for t in range(2):
                nc.tensor.matmul(pss, onesc, sq2[:, t, :],
                                 start=(t == 0), stop=(t == 1))
            nc.vector.tensor_copy(out=hp_all[0:1, 8, :], in_=pss)
            nc.gpsimd.dma_start(out=arb, in_=hp_all)

            # prefetch first MLP1 weight tiles under the collective
            w1_tiles = [None] * 32

            def load_w1(mt):
                t = ring.tile([128, KT, 128], BF16, tag="w1r", bufs=4)
                nc.sync.dma_start(out=t, in_=w1[mt])
                w1_tiles[mt] = t

            w2_tiles = [None] * 4

            def load_w2(mt):
                t = ring.tile([128, 32, 128], BF16, tag="xt", bufs=2)
                nc.sync.dma_start(out=t, in_=w2[mt])
                w2_tiles[mt] = t

            for mt in range(16, 20):
                load_w1(mt)

            arb_out = dsc.tile([128, 9, NPOS], BF16)
            nc.gpsimd.collective_compute(
                "AllReduce", mybir.AluOpType.add,
                replica_groups=[list(range(NCORES))],
                ins=[arb.opt()], outs=[arb_out.opt()],
            )
            hsum = sb.tile([128, 9, NPOS], BF16)
            nc.sync.dma_start(out=hsum, in_=arb_out)

            # r = 1/sqrt(mean + eps) = exp(-0.5*ln(mean + eps)); h = hsum*r
            eps_t = sb.tile([1, 1], F32)
            nc.vector.memset(eps_t, 1e-5)
            rs = sb.tile([1, NPOS], F32)
            nc.scalar.activation(out=rs, in_=hsum[0:1, 8, :], func=AF.Ln,
                                 bias=eps_t[:, 0:1], scale=1.0 / D_INNER)
            rsb = sb.tile([1, NPOS], BF16)
            nc.scalar.activation(out=rsb, in_=rs, func=AF.Exp, scale=-0.5)
            prs = psA.tile([128, NPOS], F32, tag="pdt", bufs=2)
            nc.tensor.matmul(prs, ones128b, rsb, start=True, stop=True)
            r_bc = sb.tile([128, NPOS], F32)
            nc.vector.tensor_copy(out=r_bc, in_=prs)
            h_sb = sb.tile([128, 8, NPOS], BF16)
            for k in range(KT):
                nc.vector.tensor_mul(h_sb[:, k, :], hsum[:, k, :], r_bc)

            # ---- g = gelu(w1T.T @ h + b1)  [4096, 32]
            g_sb = sb.tile([128, 32, NPOS], BF16)
            for mt in range(32):
                if 16 <= mt + 4 < 32 and mt + 4 >= 20:
                    load_w1(mt + 4)
                elif mt == 28:
                    load_w2(2)
                elif mt == 29:
                    load_w2(3)
                wsrc = w1pre[:, mt, :, :] if mt < 16 else w1_tiles[mt]
                pg = psA.tile([128, NPOS], F32, tag="psh", bufs=3)
                for k in range(KT):
                    nc.tensor.matmul(pg, wsrc[:, k, :],
                                     h_sb[:, k, :], start=(k == 0),
                                     stop=(k == KT - 1))
                nc.scalar.activation(out=g_sb[:, mt, :], in_=pg, func=AF.Gelu,
                                     bias=b1_sb[:, mt:mt + 1], scale=1.0)
            # ---- out = w2T.T @ g + b2   [512, 32] per core
            for mt in range(4):
                wsrc = w2pre[:, mt, :, :] if mt < 2 else w2_tiles[mt]
                po = psA.tile([128, NPOS], F32, tag="psh", bufs=3)
                for k in range(32):
                    nc.tensor.matmul(po, wsrc[:, k, :],
                                     g_sb[:, k, :], start=(k == 0), stop=(k == 31))
                ot = sb.tile([128, NPOS], F32, tag="ot", bufs=2)
                nc.vector.tensor_scalar(ot, po, b2_sb[:, mt:mt + 1], None, OP.add)
                nc.sync.dma_start(out=out32[:, mt, :], in_=ot)

    nc.compile()
    _NC = nc
    return nc


# ----------------------------------------------------------------------------
# Host-side prep + glue
# ----------------------------------------------------------------------------

def _prep_maps(inputs):
    x = _f32(inputs["x"]).reshape(L, D_MODEL)
    xT = np.ascontiguousarray(x.T)                       # [1024, 8192]
    # pre-tiled xT: [NG, 128, KT, GSZ]
    xT_t = _bf(xT.reshape(KT, 128, NG, GSZ).transpose(2, 1, 0, 3))
    pos = np.arange(NPOS) * POS_STRIDE
    xTpos = _bf(xT[:, pos].reshape(KT, 128, NPOS).transpose(1, 0, 2))
    win_idx = (pos[:, None] + np.arange(D_CONV)[None, :] - (D_CONV - 1)).reshape(-1)
    xTwin = np.zeros((D_MODEL, NPOS * D_CONV), np.float32)
    valid = win_idx >= 0
    xTwin[:, valid] = xT[:, win_idx[valid]]
    xTwin = _bf(xTwin.reshape(KT, 128, NPOS * D_CONV).transpose(1, 0, 2))

    w_all = _f32(inputs["in_proj_w"])                    # [4384, 1024]
    conv_w = _f32(inputs["conv_w"])                      # [2304, 4]
    conv_b = _f32(inputs["conv_b"])                      # [2304]
    dt_bias = _f32(inputs["dt_bias"])                    # [32]
    A = -np.exp(_f32(inputs["A_log"]))                   # [32]
    Dp = _f32(inputs["D"])                               # [32]

    w_cT = _bf(w_all[D_INNER + D_INNER + D_STATE:
                     D_INNER + D_INNER + 2 * D_STATE]
               .T.reshape(KT, 128, 128).transpose(1, 0, 2))
    cw_c = _f32(conv_w[D_INNER + D_STATE:])              # [128, 4] C channels
    conv_b_c = _f32(conv_b[D_INNER + D_STATE:]).reshape(128, 1)

    oh = np.zeros((HPC, 256), np.float32)
    for t in range(2):
        for p in range(128):
            oh[2 * t + p // 64, t * 128 + p] = 1.0
    oh = _bf(oh)

    norm_w = _f32(inputs["norm_w"])                      # [2048]
    w_out = _f32(inputs["mamba_out_w"])                  # [1024, 2048]
    w1 = _f32(inputs["mlp_w1"])                          # [4096, 1024]
    w1_t = _bf(w1.T.reshape(KT, 128, 32, 128).transpose(2, 1, 0, 3))
    b1 = _f32(inputs["mlp_b1"]).reshape(32, 128).transpose(1, 0).copy()
    w2 = _f32(inputs["mlp_w2"])                          # [4096, 4096]

    maps = []
    for k in range(NCORES):
        xs = 256 * k
        cols = np.concatenate([
            np.arange(D_INNER + xs, D_INNER + xs + 256),          # x slice
            np.arange(2 * D_INNER, 2 * D_INNER + D_STATE),        # B
        ])
        w_in = _bf(w_all[cols].T.reshape(KT, 128, NCOL).transpose(1, 0, 2))
        dt_cols = np.arange(D_IN_PROJ - NHEADS + HPC * k,
                            D_IN_PROJ - NHEADS + HPC * k + HPC)
        w_dtp = _bf(w_all[dt_cols].T.reshape(KT, 128, HPC).transpose(1, 0, 2))
        w_z = _bf(w_all[xs:xs + 256].T.reshape(KT, 128, 256).transpose(1, 0, 2))
        ch_x = np.arange(xs, xs + 256)
        ch_B = np.arange(D_INNER, D_INNER + D_STATE)
        dw = np.zeros((3, D_CONV, 128, 128), np.float32)
        cb = np.zeros((128, 3), np.float32)
        for cht, chs in enumerate([ch_x[:128], ch_x[128:], ch_B]):
            for j in range(D_CONV):
                dw[cht, j] = np.diag(conv_w[chs, j])
            cb[:, cht] = conv_b[chs]
        dw = _bf(dw.transpose(2, 0, 1, 3))               # [128, 3, 4, 128]
        heads = np.arange(HPC * k, HPC * k + HPC)
        # local channels of this core: ch = 256k + t*128 + p
        chl = (xs + np.arange(256)).reshape(2, 128)      # [t, p]
        nwl = norm_w[chl].T.copy()                       # [128, 2]
        wol = _bf(w_out[:, xs:xs + 256].T.reshape(2, 128, D_MODEL)
                  .transpose(1, 0, 2))                   # [128, 2, 1024]
        colsl = slice(512 * k, 512 * k + 512)
        w2_t = _bf(w2[colsl].T.reshape(32, 128, 4, 128).transpose(2, 1, 0, 3))
        b2 = _f32(inputs["mlp_b2"])[colsl].reshape(4, 128).transpose(1, 0).copy()
        maps.append({
            "xT": xT_t, "xTpos": xTpos, "xTwin": xTwin,
            "w_in": w_in, "w_dt": w_dtp, "w_c": w_cT, "w_z": w_z,
            "diag_w": dw, "cw_c": cw_c, "conv_b": cb,
            "conv_b_c": conv_b_c,
            "dtb4": dt_bias[heads].reshape(HPC, 1).astype(np.float32),
            "A4": A[heads].reshape(HPC, 1).astype(np.float32),
            "D4": Dp[heads].reshape(HPC, 1).astype(np.float32),
            "oh_w": oh,
            "nwl": nwl, "wol": wol, "w1T": w1_t, "b1": b1,
            "w2T": w2_t, "b2": b2,
        })
    return maps


LAST_RESULTS = []


def kernel(**inputs) -> np.ndarray:
    trace = os.environ.get("KERNEL_TRACE", "0") == "1"
    LAST_RESULTS.clear()
    nc = build()
    maps = _prep_maps(inputs)
    res = bass_utils.run_bass_kernel_spmd(nc, maps, core_ids=list(range(NCORES)),
                                          trace=trace)
    LAST_RESULTS.append(res)
    out = np.zeros((NPOS, HIDDEN), np.float32)
    for k in range(NCORES):
        o = res.results[k]["out32"]                     # [128, 4, 32]
        out[:, 512 * k:512 * (k + 1)] = o.transpose(2, 1, 0).reshape(NPOS, 512)
    return out.astype(np.float32)
